# revision 25
# baseline (speedup 1.0000x reference)
"""ALiBi sliding-window causal attention (B=2, N=2048, C=1024, H=16, D=64,
W=256) on 8 TRN2 NeuronCores.

Sharding: core = (batch b, sequence chunk c) over a 2x4 grid. Each core owns
512 queries and recomputes K/V for a 256-row halo, so the sliding-window
attention is fully local — no collectives. Matmuls run in bf16 with f32
accumulation; weights/x are pre-transposed and cast on the host.

Key trick: in the S^T = K·Q^T layout (keys on partitions), the ALiBi bias
slope_h*(j - i) splits into a per-key term (a per-partition scalar, fused into
the ScalarE exp as its bias operand) and a per-query term that is constant
along the softmax axis and therefore cancels in the normalization. The
window/causal mask is a multiplicative {0,1} tile applied by the f32->bf16
conversion multiply. The softmax denominator comes from a ones-column
appended to V.
"""

import contextlib
import math

import numpy as np
import ml_dtypes

import concourse.bass as bass
import concourse.bass_utils as bass_utils
import concourse.mybir as mybir
import concourse.tile as tile
from concourse.bass_utils import run_bass_kernel_spmd
from concourse.masks import make_identity
from concourse.vector_clock import ScopedClock

# ---------------------------------------------------------------------------
# Patch TileContext._drain_and_barrier: this container's walrus rejects >2 sem
# waits on a CTRL-class instruction ("Too many sync wait commands"), and the
# Tile kernel-tail drain aggregates one wait per live proc. Split the waits
# onto single-wait nop carriers that run just before the drain's barrier.
# ---------------------------------------------------------------------------
_MAX_DRAIN_WAITS = 1


def _patched_drain_and_barrier(self, tick_clock, wait_clock):
    nc = self.nc
    drain_inst = nc.sync.drain()
    wait_clock.add_sem_waits(
        drain_inst.ins, ScopedClock({None: tick_clock.global_clock})
    )
    si = drain_inst.ins.sync_info
    waits = list(si.on_wait) if (si is not None and si.on_wait) else []
    if len(waits) > _MAX_DRAIN_WAITS:
        ups = list(si.on_update) if (si is not None and si.on_update) else []
        drain_inst.ins.sync_info = mybir.SyncInfo(
            on_wait=waits[:_MAX_DRAIN_WAITS], on_update=ups
        )
        for i in range(_MAX_DRAIN_WAITS, len(waits), _MAX_DRAIN_WAITS):
            nop = nc.sync.nop(nofuse=True)
            nop.ins.sync_info = mybir.SyncInfo(
                on_wait=waits[i : i + _MAX_DRAIN_WAITS], on_update=[]
            )

    nc.all_engine_barrier()
    assert self.sems is not None
    popped = nc._tile_sem_poison_stack.pop()
    assert popped is self._sem_poison
    nc.clear_and_free_semaphores(list(self.sems.allocated().values()))
    nc.all_engine_barrier()


tile.TileContext._drain_and_barrier = _patched_drain_and_barrier

def _dedup_ldweights(nc: bass.Bass):
    """Tile's legalize emits one InstLdweights per matmul even when
    consecutive matmuls use the identical stationary operand. Each load costs
    ~107ns of serial PE time; drop exact-duplicate back-to-back loads (the PE
    array still holds the weights), folding any waits into the next matmul."""
    pe = mybir.EngineType.PE
    for f in nc.m.functions:
        for blk in f.blocks:
            insts = list(blk.instructions)
            new = []
            last_key = None
            pending_waits = []
            changed = False
            for inst in insts:
                if inst.engine != pe:
                    new.append(inst)
                    continue
                tn = type(inst).__name__
                if tn == "InstLdweights":
                    key = (
                        str(inst.ins[0]),
                        str(inst.tile_position),
                        str(inst.tile_size),
                        str(inst.is_transpose),
                        str(inst.perf_mode),
                    )
                    if key == last_key:
                        changed = True
                        si = inst.sync_info
                        if si is not None and si.on_wait:
                            pending_waits.extend(si.on_wait)
                        continue
                    last_key = key
                elif tn != "InstMatmult":
                    pass  # other PE insts don't touch the weight array
                if pending_waits:
                    si = inst.sync_info
                    waits = list(si.on_wait) if (si and si.on_wait) else []
                    ups = list(si.on_update) if (si and si.on_update) else []
                    inst.sync_info = mybir.SyncInfo(
                        on_wait=pending_waits + waits, on_update=ups
                    )
                    pending_waits = []
                new.append(inst)
            if changed:
                blk.instructions = new


_MAX_INST_WAITS = 1


def _split_excess_waits(nc: bass.Bass, max_waits: int = _MAX_INST_WAITS):
    """Walrus in this container rejects instructions carrying more than a
    couple of sem waits. Hoist excess waits onto same-engine nop carriers
    placed immediately before the offending instruction."""
    for f in nc.m.functions:
        for blk in f.blocks:
            snapshot = list(blk.instructions)
            new: list = []
            changed = False
            for inst in snapshot:
                si = inst.sync_info
                waits = list(si.on_wait) if (si is not None and si.on_wait) else []
                if len(waits) > max_waits:
                    changed = True
                    eng = nc.engines[inst.engine]
                    n_extra = len(waits) - max_waits
                    for i in range(0, n_extra, max_waits):
                        chunk = waits[i : min(i + max_waits, n_extra)]
                        nop = eng.nop(nofuse=True)
                        # eng.nop appended to the current bb; reclaim it
                        cur = nc.cur_bb.bb
                        cur.instructions = cur.instructions[:-1]
                        nop.ins.sync_info = mybir.SyncInfo(
                            on_wait=chunk, on_update=[]
                        )
                        new.append(nop.ins)
                    ups = list(si.on_update) if (si is not None and si.on_update) else []
                    inst.sync_info = mybir.SyncInfo(
                        on_wait=waits[n_extra:], on_update=ups
                    )
                new.append(inst)
            if changed:
                blk.instructions = new

# ---------------------------------------------------------------------------
# Problem constants (hardcoded per spec)
# ---------------------------------------------------------------------------
BF16 = ml_dtypes.bfloat16
B, N, C = 2, 2048, 1024
H, D = 16, 64
WINDOW = 256
SCALE = D ** -0.5
NCHUNK = 4  # sequence chunks per batch -> 2*4 = 8 cores
CH = N // NCHUNK  # 512 own rows per core
HALO = WINDOW  # 256 halo rows of K/V context
ROWS = CH + HALO  # 768 rows of x per core
QT_TILES = CH // 128  # 4 query tiles of 128
CBIAS = 320  # alibi per-key bias centering (overflow/underflow safe)
P = 128
KI = C // P  # 8 contraction tiles
CT3 = 3 * C // P  # 24 qkv output column tiles
VCOLS = D + 1  # per-head V columns incl. ones column
NCORES = 8


def _alibi_slopes(num_heads: int) -> np.ndarray:
    closest_pow2 = 2 ** math.floor(math.log2(num_heads))
    base = 2.0 ** (-(2.0 ** (-(math.log2(closest_pow2) - 3))))
    powers = np.arange(1, closest_pow2 + 1, dtype=np.float32)
    slopes = base ** powers
    if num_heads != closest_pow2:
        start = 2.0 ** (-(2.0 ** (-(math.log2(closest_pow2) - 3)) - 1))
        extra = np.linspace(start, base, num_heads - closest_pow2, dtype=np.float32)
        slopes = np.concatenate([slopes, extra])
    return slopes.astype(np.float32)


# ---------------------------------------------------------------------------
# Device program
# ---------------------------------------------------------------------------
def build_nc() -> bass.Bass:
    nc = bass.Bass()
    f32 = mybir.dt.float32
    bf16 = mybir.dt.bfloat16

    xt = nc.declare_dram_parameter("xt", [C, ROWS], bf16, isOutput=False)
    wt = nc.declare_dram_parameter("wt", [C, 3 * C], bf16, isOutput=False)
    pwt = nc.declare_dram_parameter("pwt", [C, C], bf16, isOutput=False)
    qkvb = nc.declare_dram_parameter("qkvb", [2 * C], f32, isOutput=False)
    vb = nc.declare_dram_parameter("vb", [C], f32, isOutput=False)
    pb = nc.declare_dram_parameter("pb", [C], f32, isOutput=False)
    mask = nc.declare_dram_parameter(
        "mask", [QT_TILES, 3, P, P], bf16, isOutput=False
    )
    ab = nc.declare_dram_parameter("ab", [H, 3, P], f32, isOutput=False)
    out = nc.declare_dram_parameter("out", [CH, C], f32, isOutput=True)

    with tile.TileContext(nc) as tc, contextlib.ExitStack() as ctx:
        consts = ctx.enter_context(tc.tile_pool(name="consts", bufs=1))
        work = ctx.enter_context(tc.tile_pool(name="work", bufs=3))
        rspool = ctx.enter_context(tc.tile_pool(name="rs", bufs=6))
        finals = ctx.enter_context(tc.tile_pool(name="finals", bufs=2))
        # one dynamic PSUM pool: every tile fits one 2KB bank, 8 banks total
        psum = ctx.enter_context(tc.tile_pool(name="psum", bufs=8, space="PSUM"))

        # ------------------------------- constant loads -------------------
        xt_sb = consts.tile([P, KI, ROWS], bf16, tag="xt")
        wt_sb = consts.tile([P, KI, 3 * C], bf16, tag="wt")
        pwt_sb = consts.tile([P, KI, C], bf16, tag="pwt")
        qkb_sb = consts.tile([P, 16], f32, tag="qkb")
        vb_sb = consts.tile([P, C], f32, tag="vb")
        pb_sb = consts.tile([P, C], f32, tag="pb")
        mask_sb = consts.tile([P, QT_TILES * 3, P], bf16, tag="mask")
        ab_sb = consts.tile([P, H * 3], f32, tag="ab")
        ident = consts.tile([P, P], bf16, tag="ident")

        xt_r = xt.rearrange("(ko p) n -> p ko n", p=P)
        wt_r = wt.rearrange("(ko p) c -> p ko c", p=P)
        pwt_r = pwt.rearrange("(ko p) c -> p ko c", p=P)
        # few big DMAs: each dma_start costs ~0.7us of SP issue time, which
        # serialized the startup when split per-ki
        nc.sync.dma_start(out=xt_sb[:], in_=xt_r[:])
        nc.sync.dma_start(out=wt_sb[:, :, 0:C], in_=wt_r[:, :, 0:C])
        nc.sync.dma_start(out=wt_sb[:, :, C : 2 * C], in_=wt_r[:, :, C : 2 * C])
        nc.sync.dma_start(out=wt_sb[:, :, 2 * C : 3 * C], in_=wt_r[:, :, 2 * C : 3 * C])
        nc.sync.dma_start(out=pwt_sb[:], in_=pwt_r[:])
        # small constants go on the gpsimd queue, off the critical SP queue
        nc.gpsimd.dma_start(out=qkb_sb[:], in_=qkvb.rearrange("(t p) -> p t", p=P))
        nc.gpsimd.dma_start(out=vb_sb[:], in_=vb[None, :].to_broadcast((P, C)))
        nc.gpsimd.dma_start(out=pb_sb[:], in_=pb[None, :].to_broadcast((P, C)))
        nc.gpsimd.dma_start(out=mask_sb[:], in_=mask.rearrange("t j p q -> p (t j) q"))
        nc.gpsimd.dma_start(out=ab_sb[:], in_=ab.rearrange("h j p -> p (h j)"))
        make_identity(nc, ident)
        # pre-warm the ScalarE Exp table (~1.3us ACT_TABLE_LOAD) off the
        # attention critical path
        warm = work.tile([P, 1], mybir.dt.float32, tag="warm")
        nc.scalar.activation(
            warm[:], qkb_sb[:, 0:1], func=mybir.ActivationFunctionType.Exp
        )

        # ------------------------------- QKV projections ------------------
        # Q^T [c_out, 512 own rows] and K^T [c_out, 768 rows]: c_out on
        # partitions (lhsT = W^T tile), rows on free dim.
        qt_sb = consts.tile([P, KI, CH], bf16, tag="qt")
        kt_sb = consts.tile([P, KI, ROWS], bf16, tag="kt")
        v_sb = consts.tile([P, ROWS // P, H * VCOLS], bf16, tag="v")

        for ct in range(KI):  # Q: c_out tiles 0..7
            ps = psum.tile([P, CH], mybir.dt.float32, tag="ps")
            for ki in range(KI):
                nc.tensor.matmul(
                    ps[:],
                    wt_sb[:, ki, ct * P : (ct + 1) * P],
                    xt_sb[:, ki, HALO:ROWS],
                    start=(ki == 0),
                    stop=(ki == KI - 1),
                )
            nc.vector.tensor_scalar_add(qt_sb[:, ct, :], ps[:], qkb_sb[:, ct : ct + 1])

        for ct in range(KI):  # K: c_out tiles 8..15
            # both row chunks inside the ki loop: adjacent matmuls share the
            # stationary W tile (one LDWEIGHTS after dedup)
            ps0 = psum.tile([P, CH], mybir.dt.float32, tag="ps")
            ps1 = psum.tile([P, CH], mybir.dt.float32, tag="ps")
            for ki in range(KI):
                w_ap = wt_sb[:, ki, C + ct * P : C + (ct + 1) * P]
                nc.tensor.matmul(
                    ps0[:],
                    w_ap,
                    xt_sb[:, ki, 0:512],
                    start=(ki == 0),
                    stop=(ki == KI - 1),
                )
                nc.tensor.matmul(
                    ps1[:, :256],
                    w_ap,
                    xt_sb[:, ki, 512:ROWS],
                    start=(ki == 0),
                    stop=(ki == KI - 1),
                )
            nc.vector.tensor_scalar_add(
                kt_sb[:, ct, 0:512], ps0[:], qkb_sb[:, KI + ct : KI + ct + 1]
            )
            nc.vector.tensor_scalar_add(
                kt_sb[:, ct, 512:ROWS], ps1[:, :256], qkb_sb[:, KI + ct : KI + ct + 1]
            )

        # V in natural [rows, c_v] layout (rows on partitions): lhsT = x^T
        # tile, rhs = W^T v-columns. Interleave a ones column per head for the
        # softmax denominator.
        for hcol in range(H):
            nc.vector.memset(v_sb[:, :, hcol * VCOLS + D : hcol * VCOLS + D + 1], 1.0)
        v_view = v_sb.rearrange("p r (h c) -> p r h c", c=VCOLS)
        for rb in range(ROWS // P):
            # both c_v chunks inside the ki loop: adjacent matmuls share the
            # stationary x^T tile (one LDWEIGHTS after dedup)
            vps = [psum.tile([P, CH], mybir.dt.float32, tag="ps", name=f"vps{_i}") for _i in range(2)]
            for ki in range(KI):
                for cc in range(2):
                    nc.tensor.matmul(
                        vps[cc][:],
                        xt_sb[:, ki, rb * P : (rb + 1) * P],
                        wt_sb[:, ki, 2 * C + cc * 512 : 2 * C + (cc + 1) * 512],
                        start=(ki == 0),
                        stop=(ki == KI - 1),
                    )
            for cc in range(2):
                nc.vector.tensor_tensor(
                    v_view[:, rb, cc * 8 : (cc + 1) * 8, 0:D],
                    vps[cc][:].rearrange("p (h c) -> p h c", c=D),
                    vb_sb[:, cc * 512 : (cc + 1) * 512].rearrange(
                        "p (h c) -> p h c", c=D
                    ),
                    mybir.AluOpType.add,
                )

        # ------------------------------- attention + proj -----------------
        for t in range(QT_TILES):
            attn_t = consts.tile([P, C], bf16, tag=f"attn_{t}")
            for hp in range(H // 2):
                # the S^T matmuls of a head pair contract on disjoint PE
                # row-groups (partitions 0-63 / 64-127); interleaving them
                # lets the PE pull each LDWEIGHTS ahead of the in-flight
                # matmul of the other head
                sts = [psum.tile([P, 3, P], mybir.dt.float32, tag="ps", name=f"sts{_i}") for _i in range(2)]
                for j in range(3):
                    for hi in range(2):
                        po = hi * 64
                        nc.tensor.matmul(
                            sts[hi][:, j, :],
                            kt_sb[po : po + 64, hp, (t + j) * P : (t + j + 1) * P],
                            qt_sb[po : po + 64, hp, t * P : (t + 1) * P],
                            start=True,
                            stop=True,
                        )
                for hi in range(2):
                    h = 2 * hp + hi
                    st_ps = sts[hi]
                    exp_t = work.tile([P, 3, P], mybir.dt.float32, tag="exp")
                    for j in range(3):
                        nc.scalar.activation(
                            exp_t[:, j, :],
                            st_ps[:, j, :],
                            func=mybir.ActivationFunctionType.Exp,
                            bias=ab_sb[:, h * 3 + j : h * 3 + j + 1],
                            scale=1.0,
                        )
                    pt = work.tile([P, 3, P], bf16, tag="pt")
                    # gpsimd is idle here and this keeps DVE (~77% busy in
                    # the attention phase) off the critical path
                    nc.gpsimd.tensor_tensor(
                        pt[:],
                        exp_t[:],
                        mask_sb[:, t * 3 : (t + 1) * 3, :],
                        mybir.AluOpType.mult,
                    )
                    o_ps = psum.tile([P, VCOLS], mybir.dt.float32, tag="ps")
                    for j in range(3):
                        nc.tensor.matmul(
                            o_ps[:],
                            pt[:, j, :],
                            v_sb[:, t + j, h * VCOLS : (h + 1) * VCOLS],
                            start=(j == 0),
                            stop=(j == 2),
                        )
                    rs = rspool.tile([P, 1], mybir.dt.float32, tag="rs")
                    nc.vector.reciprocal(rs[:], o_ps[:, D : D + 1])
                    nc.vector.tensor_scalar_mul(
                        attn_t[:, h * D : (h + 1) * D], o_ps[:, 0:D], rs[:]
                    )

            # transpose attn [q, c] -> attnT [c, q] for the output projection
            at_t = consts.tile([P, KI, P], bf16, tag=f"attnT_{t}")
            for ct in range(KI):
                tr_ps = psum.tile([P, P], bf16, tag="ps")
                nc.tensor.transpose(
                    tr_ps[:], attn_t[:, ct * P : (ct + 1) * P], ident[:]
                )
                nc.vector.tensor_copy(at_t[:, ct, :], tr_ps[:])

            fin = finals.tile([P, C], mybir.dt.float32, tag="fin")
            # both output chunks inside the ct loop: adjacent matmuls share
            # the stationary attnT tile (one LDWEIGHTS after dedup)
            pps = [psum.tile([P, CH], mybir.dt.float32, tag="ps", name=f"pps{_i}") for _i in range(2)]
            for ct in range(KI):
                for cc in range(2):
                    nc.tensor.matmul(
                        pps[cc][:],
                        at_t[:, ct, :],
                        pwt_sb[:, ct, cc * 512 : (cc + 1) * 512],
                        start=(ct == 0),
                        stop=(ct == KI - 1),
                    )
            for cc in range(2):
                nc.vector.tensor_tensor(
                    fin[:, cc * 512 : (cc + 1) * 512],
                    pps[cc][:],
                    pb_sb[:, cc * 512 : (cc + 1) * 512],
                    mybir.AluOpType.add,
                )
            nc.sync.dma_start(out=out[t * P : (t + 1) * P, :], in_=fin[:])

    _dedup_ldweights(nc)
    _split_excess_waits(nc)
    return nc


_NC_CACHE = None


def _get_nc() -> bass.Bass:
    global _NC_CACHE
    if _NC_CACHE is None:
        _NC_CACHE = build_nc()
    return _NC_CACHE


# ---------------------------------------------------------------------------
# Host side: shard, pre-transpose, cast; run SPMD; gather
# ---------------------------------------------------------------------------
def make_in_maps(x, qkv_w, qkv_b, proj_w, proj_b):
    x = np.asarray(x, np.float32)
    qkv_w = np.asarray(qkv_w, np.float32)
    qkv_b = np.asarray(qkv_b, np.float32)
    proj_w = np.asarray(proj_w, np.float32)
    proj_b = np.asarray(proj_b, np.float32)

    # fold the attention scale into the Q projection
    qkv_w = qkv_w.copy()
    qkv_b = qkv_b.copy()
    qkv_w[:C] *= SCALE
    qkv_b[:C] *= SCALE

    wt_np = np.ascontiguousarray(qkv_w.T).astype(BF16)
    pwt_np = np.ascontiguousarray(proj_w.T).astype(BF16)
    qkvb_np = np.ascontiguousarray(qkv_b[: 2 * C])
    vb_np = np.ascontiguousarray(qkv_b[2 * C :])
    pb_np = proj_b

    slopes = _alibi_slopes(H)
    jj = np.arange(3, dtype=np.float32)[:, None]
    pp = np.arange(P, dtype=np.float32)[None, :]
    ab_np = np.ascontiguousarray(
        slopes[:, None, None] * (jj * P + pp - CBIAS)[None]
    ).astype(np.float32)  # [H, 3, P]

    tt = np.arange(QT_TILES)[:, None, None, None]
    jj4 = np.arange(3)[None, :, None, None]
    kk = np.arange(P)[None, None, :, None]
    qq = np.arange(P)[None, None, None, :]
    dist = jj4 * P + kk - qq  # key_local - t*128 - q_local
    valid = (dist >= 1) & (dist <= WINDOW)
    valid0 = valid & ((tt * P + jj4 * P + kk) >= HALO)  # chunk 0: no past ctx

    mask_np = np.broadcast_to(valid.astype(BF16), (QT_TILES, 3, P, P))
    mask0_np = valid0.astype(BF16)

    in_maps = []
    for core in range(NCORES):
        b, c = divmod(core, NCHUNK)
        n0 = c * CH
        xh = np.zeros((ROWS, C), np.float32)
        lo = max(0, n0 - HALO)
        xh[HALO - (n0 - lo) :] = x[b, lo : n0 + CH]
        in_maps.append(
            {
                "xt": np.ascontiguousarray(xh.T).astype(BF16),
                "wt": wt_np,
                "pwt": pwt_np,
                "qkvb": qkvb_np,
                "vb": vb_np,
                "pb": pb_np,
                "mask": np.ascontiguousarray(mask0_np if c == 0 else mask_np),
                "ab": ab_np,
            }
        )
    return in_maps


def run(in_maps, trace=False, **kw):
    res = run_bass_kernel_spmd(
        _get_nc(), in_maps, core_ids=list(range(NCORES)), trace=trace, **kw
    )
    return res


def kernel(x, qkv_w, qkv_b, proj_w, proj_b):
    in_maps = make_in_maps(x, qkv_w, qkv_b, proj_w, proj_b)
    res = run(in_maps)
    out = np.empty((B, N, C), np.float32)
    for core in range(NCORES):
        b, c = divmod(core, NCHUNK)
        out[b, c * CH : (c + 1) * CH] = res.results[core]["out"]
    return out


# revision 26
# speedup vs baseline: 1.1242x; 1.1242x over previous
"""ALiBi sliding-window causal attention (B=2, N=2048, C=1024, H=16, D=64,
W=256) on 8 TRN2 NeuronCores.

Sharding: core = (batch b, sequence chunk c) over a 2x4 grid. Each core owns
512 queries and recomputes K/V for a 256-row halo, so the sliding-window
attention is fully local — no collectives. Matmuls run in bf16 with f32
accumulation; weights/x are pre-transposed and cast on the host.

Key trick: in the S^T = K·Q^T layout (keys on partitions), the ALiBi bias
slope_h*(j - i) splits into a per-key term (a per-partition scalar, fused into
the ScalarE exp as its bias operand) and a per-query term that is constant
along the softmax axis and therefore cancels in the normalization. The
window/causal mask is a multiplicative {0,1} tile applied by the f32->bf16
conversion multiply. The softmax denominator comes from a ones-column
appended to V.
"""

import contextlib
import math

import numpy as np
import ml_dtypes

import concourse.bass as bass
import concourse.bass_utils as bass_utils
import concourse.mybir as mybir
import concourse.tile as tile
from concourse.bass_utils import run_bass_kernel_spmd
from concourse.masks import make_identity
from concourse.vector_clock import ScopedClock

# ---------------------------------------------------------------------------
# Patch TileContext._drain_and_barrier: this container's walrus rejects >2 sem
# waits on a CTRL-class instruction ("Too many sync wait commands"), and the
# Tile kernel-tail drain aggregates one wait per live proc. Split the waits
# onto single-wait nop carriers that run just before the drain's barrier.
# ---------------------------------------------------------------------------
_MAX_DRAIN_WAITS = 1


def _patched_drain_and_barrier(self, tick_clock, wait_clock):
    nc = self.nc
    drain_inst = nc.sync.drain()
    wait_clock.add_sem_waits(
        drain_inst.ins, ScopedClock({None: tick_clock.global_clock})
    )
    si = drain_inst.ins.sync_info
    waits = list(si.on_wait) if (si is not None and si.on_wait) else []
    if len(waits) > _MAX_DRAIN_WAITS:
        ups = list(si.on_update) if (si is not None and si.on_update) else []
        drain_inst.ins.sync_info = mybir.SyncInfo(
            on_wait=waits[:_MAX_DRAIN_WAITS], on_update=ups
        )
        for i in range(_MAX_DRAIN_WAITS, len(waits), _MAX_DRAIN_WAITS):
            nop = nc.sync.nop(nofuse=True)
            nop.ins.sync_info = mybir.SyncInfo(
                on_wait=waits[i : i + _MAX_DRAIN_WAITS], on_update=[]
            )

    nc.all_engine_barrier()
    assert self.sems is not None
    popped = nc._tile_sem_poison_stack.pop()
    assert popped is self._sem_poison
    nc.clear_and_free_semaphores(list(self.sems.allocated().values()))
    nc.all_engine_barrier()


tile.TileContext._drain_and_barrier = _patched_drain_and_barrier

def _dedup_ldweights(nc: bass.Bass):
    """Tile's legalize emits one InstLdweights per matmul even when
    consecutive matmuls use the identical stationary operand. Each load costs
    ~107ns of serial PE time; drop exact-duplicate back-to-back loads (the PE
    array still holds the weights), folding any waits into the next matmul."""
    pe = mybir.EngineType.PE
    for f in nc.m.functions:
        for blk in f.blocks:
            insts = list(blk.instructions)
            new = []
            last_key = None
            pending_waits = []
            changed = False
            for inst in insts:
                if inst.engine != pe:
                    new.append(inst)
                    continue
                tn = type(inst).__name__
                if tn == "InstLdweights":
                    key = (
                        str(inst.ins[0]),
                        str(inst.tile_position),
                        str(inst.tile_size),
                        str(inst.is_transpose),
                        str(inst.perf_mode),
                    )
                    if key == last_key:
                        changed = True
                        si = inst.sync_info
                        if si is not None and si.on_wait:
                            pending_waits.extend(si.on_wait)
                        continue
                    last_key = key
                elif tn != "InstMatmult":
                    pass  # other PE insts don't touch the weight array
                if pending_waits:
                    si = inst.sync_info
                    waits = list(si.on_wait) if (si and si.on_wait) else []
                    ups = list(si.on_update) if (si and si.on_update) else []
                    inst.sync_info = mybir.SyncInfo(
                        on_wait=pending_waits + waits, on_update=ups
                    )
                    pending_waits = []
                new.append(inst)
            if changed:
                blk.instructions = new


_MAX_INST_WAITS = 1


def _split_excess_waits(nc: bass.Bass, max_waits: int = _MAX_INST_WAITS):
    """Walrus in this container rejects instructions carrying more than a
    couple of sem waits. Hoist excess waits onto same-engine nop carriers
    placed immediately before the offending instruction."""
    for f in nc.m.functions:
        for blk in f.blocks:
            snapshot = list(blk.instructions)
            new: list = []
            changed = False
            for inst in snapshot:
                si = inst.sync_info
                waits = list(si.on_wait) if (si is not None and si.on_wait) else []
                if len(waits) > max_waits:
                    changed = True
                    eng = nc.engines[inst.engine]
                    n_extra = len(waits) - max_waits
                    for i in range(0, n_extra, max_waits):
                        chunk = waits[i : min(i + max_waits, n_extra)]
                        nop = eng.nop(nofuse=True)
                        # eng.nop appended to the current bb; reclaim it
                        cur = nc.cur_bb.bb
                        cur.instructions = cur.instructions[:-1]
                        nop.ins.sync_info = mybir.SyncInfo(
                            on_wait=chunk, on_update=[]
                        )
                        new.append(nop.ins)
                    ups = list(si.on_update) if (si is not None and si.on_update) else []
                    inst.sync_info = mybir.SyncInfo(
                        on_wait=waits[n_extra:], on_update=ups
                    )
                new.append(inst)
            if changed:
                blk.instructions = new

# ---------------------------------------------------------------------------
# Problem constants (hardcoded per spec)
# ---------------------------------------------------------------------------
BF16 = ml_dtypes.bfloat16
B, N, C = 2, 2048, 1024
H, D = 16, 64
WINDOW = 256
SCALE = D ** -0.5
NCHUNK = 4  # sequence chunks per batch -> 2*4 = 8 cores
CH = N // NCHUNK  # 512 own rows per core
HALO = WINDOW  # 256 halo rows of K/V context
ROWS = CH + HALO  # 768 rows of x per core
QT_TILES = CH // 128  # 4 query tiles of 128
CBIAS = 320  # alibi per-key bias centering (overflow/underflow safe)
P = 128
KI = C // P  # 8 contraction tiles
CT3 = 3 * C // P  # 24 qkv output column tiles
VCOLS = D + 1  # per-head V columns incl. ones column
NCORES = 8


def _alibi_slopes(num_heads: int) -> np.ndarray:
    closest_pow2 = 2 ** math.floor(math.log2(num_heads))
    base = 2.0 ** (-(2.0 ** (-(math.log2(closest_pow2) - 3))))
    powers = np.arange(1, closest_pow2 + 1, dtype=np.float32)
    slopes = base ** powers
    if num_heads != closest_pow2:
        start = 2.0 ** (-(2.0 ** (-(math.log2(closest_pow2) - 3)) - 1))
        extra = np.linspace(start, base, num_heads - closest_pow2, dtype=np.float32)
        slopes = np.concatenate([slopes, extra])
    return slopes.astype(np.float32)


# ---------------------------------------------------------------------------
# Device program
# ---------------------------------------------------------------------------
def build_nc() -> bass.Bass:
    nc = bass.Bass()
    f32 = mybir.dt.float32
    bf16 = mybir.dt.bfloat16

    xt = nc.declare_dram_parameter("xt", [C, ROWS], bf16, isOutput=False)
    wt = nc.declare_dram_parameter("wt", [C, 3 * C], bf16, isOutput=False)
    pwt = nc.declare_dram_parameter("pwt", [C, C], bf16, isOutput=False)
    qkvb = nc.declare_dram_parameter("qkvb", [2 * C], f32, isOutput=False)
    vb = nc.declare_dram_parameter("vb", [C], f32, isOutput=False)
    pb = nc.declare_dram_parameter("pb", [C], f32, isOutput=False)
    mask = nc.declare_dram_parameter(
        "mask", [QT_TILES, 3, P, P], bf16, isOutput=False
    )
    ab = nc.declare_dram_parameter("ab", [H, 3, P], f32, isOutput=False)
    out = nc.declare_dram_parameter("out", [CH, C], f32, isOutput=True)

    with tile.TileContext(nc) as tc, contextlib.ExitStack() as ctx:
        consts = ctx.enter_context(tc.tile_pool(name="consts", bufs=1))
        work = ctx.enter_context(tc.tile_pool(name="work", bufs=3))
        rspool = ctx.enter_context(tc.tile_pool(name="rs", bufs=6))
        finals = ctx.enter_context(tc.tile_pool(name="finals", bufs=2))
        # one dynamic PSUM pool: every tile fits one 2KB bank, 8 banks total
        psum = ctx.enter_context(tc.tile_pool(name="psum", bufs=8, space="PSUM"))

        # ------------------------------- constant loads -------------------
        xt_sb = consts.tile([P, KI, ROWS], bf16, tag="xt")
        wt_sb = consts.tile([P, KI, 3 * C], bf16, tag="wt")
        pwt_sb = consts.tile([P, KI, C], bf16, tag="pwt")
        qkb_sb = consts.tile([P, 16], f32, tag="qkb")
        vb_sb = consts.tile([P, C], f32, tag="vb")
        pb_sb = consts.tile([P, C], f32, tag="pb")
        mask_sb = consts.tile([P, QT_TILES * 3, P], bf16, tag="mask")
        ab_sb = consts.tile([P, H * 3], f32, tag="ab")
        ident = consts.tile([P, P], bf16, tag="ident")

        xt_r = xt.rearrange("(ko p) n -> p ko n", p=P)
        wt_r = wt.rearrange("(ko p) c -> p ko c", p=P)
        pwt_r = pwt.rearrange("(ko p) c -> p ko c", p=P)
        # few big DMAs: each dma_start costs ~0.7us of SP issue time, which
        # serialized the startup when split per-ki
        nc.sync.dma_start(out=xt_sb[:], in_=xt_r[:])
        nc.sync.dma_start(out=wt_sb[:, :, 0:C], in_=wt_r[:, :, 0:C])
        nc.sync.dma_start(out=wt_sb[:, :, C : 2 * C], in_=wt_r[:, :, C : 2 * C])
        nc.sync.dma_start(out=wt_sb[:, :, 2 * C : 3 * C], in_=wt_r[:, :, 2 * C : 3 * C])
        nc.sync.dma_start(out=pwt_sb[:], in_=pwt_r[:])
        # small constants go on the gpsimd queue, off the critical SP queue
        nc.gpsimd.dma_start(out=qkb_sb[:], in_=qkvb.rearrange("(t p) -> p t", p=P))
        nc.gpsimd.dma_start(out=vb_sb[:], in_=vb[None, :].to_broadcast((P, C)))
        nc.gpsimd.dma_start(out=pb_sb[:], in_=pb[None, :].to_broadcast((P, C)))
        nc.gpsimd.dma_start(out=mask_sb[:], in_=mask.rearrange("t j p q -> p (t j) q"))
        nc.gpsimd.dma_start(out=ab_sb[:], in_=ab.rearrange("h j p -> p (h j)"))
        make_identity(nc, ident)
        # pre-warm the ScalarE Exp table (~1.3us ACT_TABLE_LOAD) off the
        # attention critical path
        warm = work.tile([P, 1], mybir.dt.float32, tag="warm")
        nc.scalar.activation(
            warm[:], qkb_sb[:, 0:1], func=mybir.ActivationFunctionType.Exp
        )

        # ------------------------------- QKV projections ------------------
        # Q^T [c_out, 512 own rows] and K^T [c_out, 768 rows]: c_out on
        # partitions (lhsT = W^T tile), rows on free dim.
        qt_sb = consts.tile([P, KI, CH], bf16, tag="qt")
        kt_sb = consts.tile([P, KI, ROWS], bf16, tag="kt")
        v_sb = consts.tile([P, ROWS // P, H * VCOLS], bf16, tag="v")

        for ct in range(KI):  # Q: c_out tiles 0..7
            ps = psum.tile([P, CH], mybir.dt.float32, tag="ps")
            for ki in range(KI):
                nc.tensor.matmul(
                    ps[:],
                    wt_sb[:, ki, ct * P : (ct + 1) * P],
                    xt_sb[:, ki, HALO:ROWS],
                    start=(ki == 0),
                    stop=(ki == KI - 1),
                )
            nc.vector.tensor_scalar_add(qt_sb[:, ct, :], ps[:], qkb_sb[:, ct : ct + 1])

        for ct in range(KI):  # K: c_out tiles 8..15
            # both row chunks inside the ki loop: adjacent matmuls share the
            # stationary W tile (one LDWEIGHTS after dedup)
            ps0 = psum.tile([P, CH], mybir.dt.float32, tag="ps")
            ps1 = psum.tile([P, CH], mybir.dt.float32, tag="ps")
            for ki in range(KI):
                w_ap = wt_sb[:, ki, C + ct * P : C + (ct + 1) * P]
                nc.tensor.matmul(
                    ps0[:],
                    w_ap,
                    xt_sb[:, ki, 0:512],
                    start=(ki == 0),
                    stop=(ki == KI - 1),
                )
                nc.tensor.matmul(
                    ps1[:, :256],
                    w_ap,
                    xt_sb[:, ki, 512:ROWS],
                    start=(ki == 0),
                    stop=(ki == KI - 1),
                )
            nc.vector.tensor_scalar_add(
                kt_sb[:, ct, 0:512], ps0[:], qkb_sb[:, KI + ct : KI + ct + 1]
            )
            nc.vector.tensor_scalar_add(
                kt_sb[:, ct, 512:ROWS], ps1[:, :256], qkb_sb[:, KI + ct : KI + ct + 1]
            )

        # V in natural [rows, c_v] layout (rows on partitions): lhsT = x^T
        # tile, rhs = W^T v-columns. Interleave a ones column per head for the
        # softmax denominator.
        for hcol in range(H):
            nc.vector.memset(v_sb[:, :, hcol * VCOLS + D : hcol * VCOLS + D + 1], 1.0)
        v_view = v_sb.rearrange("p r (h c) -> p r h c", c=VCOLS)
        for rb in range(ROWS // P):
            # both c_v chunks inside the ki loop: adjacent matmuls share the
            # stationary x^T tile (one LDWEIGHTS after dedup)
            vps = [psum.tile([P, CH], mybir.dt.float32, tag="ps", name=f"vps{_i}") for _i in range(2)]
            for ki in range(KI):
                for cc in range(2):
                    nc.tensor.matmul(
                        vps[cc][:],
                        xt_sb[:, ki, rb * P : (rb + 1) * P],
                        wt_sb[:, ki, 2 * C + cc * 512 : 2 * C + (cc + 1) * 512],
                        start=(ki == 0),
                        stop=(ki == KI - 1),
                    )
            for cc in range(2):
                nc.vector.tensor_tensor(
                    v_view[:, rb, cc * 8 : (cc + 1) * 8, 0:D],
                    vps[cc][:].rearrange("p (h c) -> p h c", c=D),
                    vb_sb[:, cc * 512 : (cc + 1) * 512].rearrange(
                        "p (h c) -> p h c", c=D
                    ),
                    mybir.AluOpType.add,
                )

        # ------------------------------- attention + proj -----------------
        for t in range(QT_TILES):
            attn_t = consts.tile([P, C], bf16, tag=f"attn_{t}")
            for hp in range(H // 2):
                # the S^T matmuls of a head pair contract on disjoint PE
                # row-groups (partitions 0-63 / 64-127); interleaving them
                # lets the PE pull each LDWEIGHTS ahead of the in-flight
                # matmul of the other head
                sts = [psum.tile([P, 3, P], mybir.dt.float32, tag="ps", name=f"sts{_i}") for _i in range(2)]
                for j in range(3):
                    for hi in range(2):
                        po = hi * 64
                        nc.tensor.matmul(
                            sts[hi][:, j, :],
                            kt_sb[po : po + 64, hp, (t + j) * P : (t + j + 1) * P],
                            qt_sb[po : po + 64, hp, t * P : (t + 1) * P],
                            start=True,
                            stop=True,
                        )
                for hi in range(2):
                    h = 2 * hp + hi
                    st_ps = sts[hi]
                    exp_t = work.tile([P, 3, P], mybir.dt.float32, tag="exp")
                    for j in range(3):
                        nc.scalar.activation(
                            exp_t[:, j, :],
                            st_ps[:, j, :],
                            func=mybir.ActivationFunctionType.Exp,
                            bias=ab_sb[:, h * 3 + j : h * 3 + j + 1],
                            scale=1.0,
                        )
                    pt = work.tile([P, 3, P], bf16, tag="pt")
                    nc.vector.tensor_tensor(
                        pt[:],
                        exp_t[:],
                        mask_sb[:, t * 3 : (t + 1) * 3, :],
                        mybir.AluOpType.mult,
                    )
                    o_ps = psum.tile([P, VCOLS], mybir.dt.float32, tag="ps")
                    for j in range(3):
                        nc.tensor.matmul(
                            o_ps[:],
                            pt[:, j, :],
                            v_sb[:, t + j, h * VCOLS : (h + 1) * VCOLS],
                            start=(j == 0),
                            stop=(j == 2),
                        )
                    rs = rspool.tile([P, 1], mybir.dt.float32, tag="rs")
                    nc.vector.reciprocal(rs[:], o_ps[:, D : D + 1])
                    nc.vector.tensor_scalar_mul(
                        attn_t[:, h * D : (h + 1) * D], o_ps[:, 0:D], rs[:]
                    )

            # transpose attn [q, c] -> attnT [c, q] for the output projection
            at_t = consts.tile([P, KI, P], bf16, tag=f"attnT_{t}")
            for ct in range(KI):
                tr_ps = psum.tile([P, P], bf16, tag="ps")
                nc.tensor.transpose(
                    tr_ps[:], attn_t[:, ct * P : (ct + 1) * P], ident[:]
                )
                nc.vector.tensor_copy(at_t[:, ct, :], tr_ps[:])

            fin = finals.tile([P, C], mybir.dt.float32, tag="fin")
            # both output chunks inside the ct loop: adjacent matmuls share
            # the stationary attnT tile (one LDWEIGHTS after dedup)
            pps = [psum.tile([P, CH], mybir.dt.float32, tag="ps", name=f"pps{_i}") for _i in range(2)]
            for ct in range(KI):
                for cc in range(2):
                    nc.tensor.matmul(
                        pps[cc][:],
                        at_t[:, ct, :],
                        pwt_sb[:, ct, cc * 512 : (cc + 1) * 512],
                        start=(ct == 0),
                        stop=(ct == KI - 1),
                    )
            for cc in range(2):
                nc.vector.tensor_tensor(
                    fin[:, cc * 512 : (cc + 1) * 512],
                    pps[cc][:],
                    pb_sb[:, cc * 512 : (cc + 1) * 512],
                    mybir.AluOpType.add,
                )
            nc.sync.dma_start(out=out[t * P : (t + 1) * P, :], in_=fin[:])

    _dedup_ldweights(nc)
    _split_excess_waits(nc)
    return nc


_NC_CACHE = None


def _get_nc() -> bass.Bass:
    global _NC_CACHE
    if _NC_CACHE is None:
        _NC_CACHE = build_nc()
    return _NC_CACHE


# ---------------------------------------------------------------------------
# Host side: shard, pre-transpose, cast; run SPMD; gather
# ---------------------------------------------------------------------------
def make_in_maps(x, qkv_w, qkv_b, proj_w, proj_b):
    x = np.asarray(x, np.float32)
    qkv_w = np.asarray(qkv_w, np.float32)
    qkv_b = np.asarray(qkv_b, np.float32)
    proj_w = np.asarray(proj_w, np.float32)
    proj_b = np.asarray(proj_b, np.float32)

    # fold the attention scale into the Q projection
    qkv_w = qkv_w.copy()
    qkv_b = qkv_b.copy()
    qkv_w[:C] *= SCALE
    qkv_b[:C] *= SCALE

    wt_np = np.ascontiguousarray(qkv_w.T).astype(BF16)
    pwt_np = np.ascontiguousarray(proj_w.T).astype(BF16)
    qkvb_np = np.ascontiguousarray(qkv_b[: 2 * C])
    vb_np = np.ascontiguousarray(qkv_b[2 * C :])
    pb_np = proj_b

    slopes = _alibi_slopes(H)
    jj = np.arange(3, dtype=np.float32)[:, None]
    pp = np.arange(P, dtype=np.float32)[None, :]
    ab_np = np.ascontiguousarray(
        slopes[:, None, None] * (jj * P + pp - CBIAS)[None]
    ).astype(np.float32)  # [H, 3, P]

    tt = np.arange(QT_TILES)[:, None, None, None]
    jj4 = np.arange(3)[None, :, None, None]
    kk = np.arange(P)[None, None, :, None]
    qq = np.arange(P)[None, None, None, :]
    dist = jj4 * P + kk - qq  # key_local - t*128 - q_local
    valid = (dist >= 1) & (dist <= WINDOW)
    valid0 = valid & ((tt * P + jj4 * P + kk) >= HALO)  # chunk 0: no past ctx

    mask_np = np.broadcast_to(valid.astype(BF16), (QT_TILES, 3, P, P))
    mask0_np = valid0.astype(BF16)

    in_maps = []
    for core in range(NCORES):
        b, c = divmod(core, NCHUNK)
        n0 = c * CH
        xh = np.zeros((ROWS, C), np.float32)
        lo = max(0, n0 - HALO)
        xh[HALO - (n0 - lo) :] = x[b, lo : n0 + CH]
        in_maps.append(
            {
                "xt": np.ascontiguousarray(xh.T).astype(BF16),
                "wt": wt_np,
                "pwt": pwt_np,
                "qkvb": qkvb_np,
                "vb": vb_np,
                "pb": pb_np,
                "mask": np.ascontiguousarray(mask0_np if c == 0 else mask_np),
                "ab": ab_np,
            }
        )
    return in_maps


def run(in_maps, trace=False, **kw):
    res = run_bass_kernel_spmd(
        _get_nc(), in_maps, core_ids=list(range(NCORES)), trace=trace, **kw
    )
    return res


def kernel(x, qkv_w, qkv_b, proj_w, proj_b):
    in_maps = make_in_maps(x, qkv_w, qkv_b, proj_w, proj_b)
    res = run(in_maps)
    out = np.empty((B, N, C), np.float32)
    for core in range(NCORES):
        b, c = divmod(core, NCHUNK)
        out[b, c * CH : (c + 1) * CH] = res.results[core]["out"]
    return out


# revision 27
# speedup vs baseline: 1.1862x; 1.0552x over previous
"""ALiBi sliding-window causal attention (B=2, N=2048, C=1024, H=16, D=64,
W=256) on 8 TRN2 NeuronCores.

Sharding: core = (batch b, sequence chunk c) over a 2x4 grid. Each core owns
512 queries and recomputes K/V for a 256-row halo, so the sliding-window
attention is fully local — no collectives. Matmuls run in bf16 with f32
accumulation; weights/x are pre-transposed and cast on the host.

Key trick: in the S^T = K·Q^T layout (keys on partitions), the ALiBi bias
slope_h*(j - i) splits into a per-key term (a per-partition scalar, fused into
the ScalarE exp as its bias operand) and a per-query term that is constant
along the softmax axis and therefore cancels in the normalization. The
window/causal mask is a multiplicative {0,1} tile applied by the f32->bf16
conversion multiply. The softmax denominator comes from a ones-column
appended to V.
"""

import contextlib
import math

import numpy as np
import ml_dtypes

import concourse.bass as bass
import concourse.bass_utils as bass_utils
import concourse.mybir as mybir
import concourse.tile as tile
from concourse.bass_utils import run_bass_kernel_spmd
from concourse.masks import make_identity
from concourse.vector_clock import ScopedClock

# ---------------------------------------------------------------------------
# Patch TileContext._drain_and_barrier: this container's walrus rejects >2 sem
# waits on a CTRL-class instruction ("Too many sync wait commands"), and the
# Tile kernel-tail drain aggregates one wait per live proc. Split the waits
# onto single-wait nop carriers that run just before the drain's barrier.
# ---------------------------------------------------------------------------
_MAX_DRAIN_WAITS = 1


def _patched_drain_and_barrier(self, tick_clock, wait_clock):
    nc = self.nc
    drain_inst = nc.sync.drain()
    wait_clock.add_sem_waits(
        drain_inst.ins, ScopedClock({None: tick_clock.global_clock})
    )
    si = drain_inst.ins.sync_info
    waits = list(si.on_wait) if (si is not None and si.on_wait) else []
    if len(waits) > _MAX_DRAIN_WAITS:
        ups = list(si.on_update) if (si is not None and si.on_update) else []
        drain_inst.ins.sync_info = mybir.SyncInfo(
            on_wait=waits[:_MAX_DRAIN_WAITS], on_update=ups
        )
        for i in range(_MAX_DRAIN_WAITS, len(waits), _MAX_DRAIN_WAITS):
            nop = nc.sync.nop(nofuse=True)
            nop.ins.sync_info = mybir.SyncInfo(
                on_wait=waits[i : i + _MAX_DRAIN_WAITS], on_update=[]
            )

    nc.all_engine_barrier()
    assert self.sems is not None
    popped = nc._tile_sem_poison_stack.pop()
    assert popped is self._sem_poison
    nc.clear_and_free_semaphores(list(self.sems.allocated().values()))
    nc.all_engine_barrier()


tile.TileContext._drain_and_barrier = _patched_drain_and_barrier

def _dedup_ldweights(nc: bass.Bass):
    """Tile's legalize emits one InstLdweights per matmul even when
    consecutive matmuls use the identical stationary operand. Each load costs
    ~107ns of serial PE time; drop exact-duplicate back-to-back loads (the PE
    array still holds the weights), folding any waits into the next matmul."""
    pe = mybir.EngineType.PE
    for f in nc.m.functions:
        for blk in f.blocks:
            insts = list(blk.instructions)
            new = []
            last_key = None
            pending_waits = []
            changed = False
            for inst in insts:
                if inst.engine != pe:
                    new.append(inst)
                    continue
                tn = type(inst).__name__
                if tn == "InstLdweights":
                    key = (
                        str(inst.ins[0]),
                        str(inst.tile_position),
                        str(inst.tile_size),
                        str(inst.is_transpose),
                        str(inst.perf_mode),
                    )
                    if key == last_key:
                        changed = True
                        si = inst.sync_info
                        if si is not None and si.on_wait:
                            pending_waits.extend(si.on_wait)
                        continue
                    last_key = key
                elif tn != "InstMatmult":
                    pass  # other PE insts don't touch the weight array
                if pending_waits:
                    si = inst.sync_info
                    waits = list(si.on_wait) if (si and si.on_wait) else []
                    ups = list(si.on_update) if (si and si.on_update) else []
                    inst.sync_info = mybir.SyncInfo(
                        on_wait=pending_waits + waits, on_update=ups
                    )
                    pending_waits = []
                new.append(inst)
            if changed:
                blk.instructions = new


_MAX_INST_WAITS = 1


def _split_excess_waits(nc: bass.Bass, max_waits: int = _MAX_INST_WAITS):
    """Walrus in this container rejects instructions carrying more than a
    couple of sem waits. Hoist excess waits onto same-engine nop carriers
    placed immediately before the offending instruction."""
    for f in nc.m.functions:
        for blk in f.blocks:
            snapshot = list(blk.instructions)
            new: list = []
            changed = False
            for inst in snapshot:
                si = inst.sync_info
                waits = list(si.on_wait) if (si is not None and si.on_wait) else []
                if len(waits) > max_waits:
                    changed = True
                    eng = nc.engines[inst.engine]
                    n_extra = len(waits) - max_waits
                    for i in range(0, n_extra, max_waits):
                        chunk = waits[i : min(i + max_waits, n_extra)]
                        nop = eng.nop(nofuse=True)
                        # eng.nop appended to the current bb; reclaim it
                        cur = nc.cur_bb.bb
                        cur.instructions = cur.instructions[:-1]
                        nop.ins.sync_info = mybir.SyncInfo(
                            on_wait=chunk, on_update=[]
                        )
                        new.append(nop.ins)
                    ups = list(si.on_update) if (si is not None and si.on_update) else []
                    inst.sync_info = mybir.SyncInfo(
                        on_wait=waits[n_extra:], on_update=ups
                    )
                new.append(inst)
            if changed:
                blk.instructions = new

# ---------------------------------------------------------------------------
# Problem constants (hardcoded per spec)
# ---------------------------------------------------------------------------
BF16 = ml_dtypes.bfloat16
B, N, C = 2, 2048, 1024
H, D = 16, 64
WINDOW = 256
SCALE = D ** -0.5
NCHUNK = 4  # sequence chunks per batch -> 2*4 = 8 cores
CH = N // NCHUNK  # 512 own rows per core
HALO = WINDOW  # 256 halo rows of K/V context
ROWS = CH + HALO  # 768 rows of x per core
QT_TILES = CH // 128  # 4 query tiles of 128
CBIAS = 320  # alibi per-key bias centering (overflow/underflow safe)
P = 128
KI = C // P  # 8 contraction tiles
CT3 = 3 * C // P  # 24 qkv output column tiles
VCOLS = D + 1  # per-head V columns incl. ones column
NCORES = 8


def _alibi_slopes(num_heads: int) -> np.ndarray:
    closest_pow2 = 2 ** math.floor(math.log2(num_heads))
    base = 2.0 ** (-(2.0 ** (-(math.log2(closest_pow2) - 3))))
    powers = np.arange(1, closest_pow2 + 1, dtype=np.float32)
    slopes = base ** powers
    if num_heads != closest_pow2:
        start = 2.0 ** (-(2.0 ** (-(math.log2(closest_pow2) - 3)) - 1))
        extra = np.linspace(start, base, num_heads - closest_pow2, dtype=np.float32)
        slopes = np.concatenate([slopes, extra])
    return slopes.astype(np.float32)


# ---------------------------------------------------------------------------
# Device program
# ---------------------------------------------------------------------------
def build_nc() -> bass.Bass:
    nc = bass.Bass()
    f32 = mybir.dt.float32
    bf16 = mybir.dt.bfloat16

    xt = nc.declare_dram_parameter("xt", [C, ROWS], bf16, isOutput=False)
    wt = nc.declare_dram_parameter("wt", [C, 3 * C], bf16, isOutput=False)
    pwt = nc.declare_dram_parameter("pwt", [C, C], bf16, isOutput=False)
    qkvb = nc.declare_dram_parameter("qkvb", [2 * C], f32, isOutput=False)
    vb = nc.declare_dram_parameter("vb", [C], f32, isOutput=False)
    pb = nc.declare_dram_parameter("pb", [C], f32, isOutput=False)
    mask = nc.declare_dram_parameter(
        "mask", [QT_TILES, 3, P, P], bf16, isOutput=False
    )
    ab = nc.declare_dram_parameter("ab", [H, 3, P], f32, isOutput=False)
    out = nc.declare_dram_parameter("out", [CH, C], f32, isOutput=True)

    with tile.TileContext(nc) as tc, contextlib.ExitStack() as ctx:
        consts = ctx.enter_context(tc.tile_pool(name="consts", bufs=1))
        work = ctx.enter_context(tc.tile_pool(name="work", bufs=3))
        rspool = ctx.enter_context(tc.tile_pool(name="rs", bufs=6))
        finals = ctx.enter_context(tc.tile_pool(name="finals", bufs=2))
        # one dynamic PSUM pool: every tile fits one 2KB bank, 8 banks total
        psum = ctx.enter_context(tc.tile_pool(name="psum", bufs=8, space="PSUM"))

        # ------------------------------- constant loads -------------------
        xt_sb = consts.tile([P, KI, ROWS], bf16, tag="xt")
        wt_sb = consts.tile([P, KI, 3 * C], bf16, tag="wt")
        pwt_sb = consts.tile([P, KI, C], bf16, tag="pwt")
        qkb_sb = consts.tile([P, 16], f32, tag="qkb")
        vb_sb = consts.tile([P, C], f32, tag="vb")
        pb_sb = consts.tile([P, C], f32, tag="pb")
        mask_sb = consts.tile([P, QT_TILES * 3, P], bf16, tag="mask")
        ab_sb = consts.tile([P, H * 3], f32, tag="ab")
        ident = consts.tile([P, P], bf16, tag="ident")

        xt_r = xt.rearrange("(ko p) n -> p ko n", p=P)
        wt_r = wt.rearrange("(ko p) c -> p ko c", p=P)
        pwt_r = pwt.rearrange("(ko p) c -> p ko c", p=P)
        for ki in range(KI):
            # Q weights + x interleaved per-ki so the Q matmul pipeline can
            # start as soon as the first ki chunk lands
            nc.sync.dma_start(out=wt_sb[:, ki, 0:C], in_=wt_r[:, ki, 0:C])
            nc.sync.dma_start(out=xt_sb[:, ki, :], in_=xt_r[:, ki, :])
        for ki in range(KI):
            nc.sync.dma_start(out=wt_sb[:, ki, C : 2 * C], in_=wt_r[:, ki, C : 2 * C])
        for ki in range(KI):
            nc.sync.dma_start(
                out=wt_sb[:, ki, 2 * C : 3 * C], in_=wt_r[:, ki, 2 * C : 3 * C]
            )
        for ki in range(KI):
            nc.sync.dma_start(out=pwt_sb[:, ki, :], in_=pwt_r[:, ki, :])
        # small constants go on the gpsimd queue, off the critical SP queue
        nc.gpsimd.dma_start(out=qkb_sb[:], in_=qkvb.rearrange("(t p) -> p t", p=P))
        nc.gpsimd.dma_start(out=vb_sb[:], in_=vb[None, :].to_broadcast((P, C)))
        nc.gpsimd.dma_start(out=pb_sb[:], in_=pb[None, :].to_broadcast((P, C)))
        nc.gpsimd.dma_start(out=mask_sb[:], in_=mask.rearrange("t j p q -> p (t j) q"))
        nc.gpsimd.dma_start(out=ab_sb[:], in_=ab.rearrange("h j p -> p (h j)"))
        make_identity(nc, ident)
        # pre-warm the ScalarE Exp table (~1.3us ACT_TABLE_LOAD) off the
        # attention critical path
        warm = work.tile([P, 1], mybir.dt.float32, tag="warm")
        nc.scalar.activation(
            warm[:], qkb_sb[:, 0:1], func=mybir.ActivationFunctionType.Exp
        )

        # ------------------------------- QKV projections ------------------
        # Q^T [c_out, 512 own rows] and K^T [c_out, 768 rows]: c_out on
        # partitions (lhsT = W^T tile), rows on free dim.
        qt_sb = consts.tile([P, KI, CH], bf16, tag="qt")
        kt_sb = consts.tile([P, KI, ROWS], bf16, tag="kt")
        v_sb = consts.tile([P, ROWS // P, H * VCOLS], bf16, tag="v")

        for ct in range(KI):  # Q: c_out tiles 0..7
            ps = psum.tile([P, CH], mybir.dt.float32, tag="ps")
            for ki in range(KI):
                nc.tensor.matmul(
                    ps[:],
                    wt_sb[:, ki, ct * P : (ct + 1) * P],
                    xt_sb[:, ki, HALO:ROWS],
                    start=(ki == 0),
                    stop=(ki == KI - 1),
                )
            nc.vector.tensor_scalar_add(qt_sb[:, ct, :], ps[:], qkb_sb[:, ct : ct + 1])

        for ct in range(KI):  # K: c_out tiles 8..15
            # both row chunks inside the ki loop: adjacent matmuls share the
            # stationary W tile (one LDWEIGHTS after dedup)
            ps0 = psum.tile([P, CH], mybir.dt.float32, tag="ps")
            ps1 = psum.tile([P, CH], mybir.dt.float32, tag="ps")
            for ki in range(KI):
                w_ap = wt_sb[:, ki, C + ct * P : C + (ct + 1) * P]
                nc.tensor.matmul(
                    ps0[:],
                    w_ap,
                    xt_sb[:, ki, 0:512],
                    start=(ki == 0),
                    stop=(ki == KI - 1),
                )
                nc.tensor.matmul(
                    ps1[:, :256],
                    w_ap,
                    xt_sb[:, ki, 512:ROWS],
                    start=(ki == 0),
                    stop=(ki == KI - 1),
                )
            nc.vector.tensor_scalar_add(
                kt_sb[:, ct, 0:512], ps0[:], qkb_sb[:, KI + ct : KI + ct + 1]
            )
            nc.vector.tensor_scalar_add(
                kt_sb[:, ct, 512:ROWS], ps1[:, :256], qkb_sb[:, KI + ct : KI + ct + 1]
            )

        # V in natural [rows, c_v] layout (rows on partitions): lhsT = x^T
        # tile, rhs = W^T v-columns. Interleave a ones column per head for the
        # softmax denominator.
        for hcol in range(H):
            nc.vector.memset(v_sb[:, :, hcol * VCOLS + D : hcol * VCOLS + D + 1], 1.0)
        v_view = v_sb.rearrange("p r (h c) -> p r h c", c=VCOLS)
        for rb in range(ROWS // P):
            # both c_v chunks inside the ki loop: adjacent matmuls share the
            # stationary x^T tile (one LDWEIGHTS after dedup)
            vps = [psum.tile([P, CH], mybir.dt.float32, tag="ps", name=f"vps{_i}") for _i in range(2)]
            for ki in range(KI):
                for cc in range(2):
                    nc.tensor.matmul(
                        vps[cc][:],
                        xt_sb[:, ki, rb * P : (rb + 1) * P],
                        wt_sb[:, ki, 2 * C + cc * 512 : 2 * C + (cc + 1) * 512],
                        start=(ki == 0),
                        stop=(ki == KI - 1),
                    )
            for cc in range(2):
                nc.vector.tensor_tensor(
                    v_view[:, rb, cc * 8 : (cc + 1) * 8, 0:D],
                    vps[cc][:].rearrange("p (h c) -> p h c", c=D),
                    vb_sb[:, cc * 512 : (cc + 1) * 512].rearrange(
                        "p (h c) -> p h c", c=D
                    ),
                    mybir.AluOpType.add,
                )

        # ------------------------------- attention + proj -----------------
        for t in range(QT_TILES):
            attn_t = consts.tile([P, C], bf16, tag=f"attn_{t}")
            for hp in range(H // 2):
                # the S^T matmuls of a head pair contract on disjoint PE
                # row-groups (partitions 0-63 / 64-127); interleaving them
                # lets the PE pull each LDWEIGHTS ahead of the in-flight
                # matmul of the other head
                sts = [psum.tile([P, 3, P], mybir.dt.float32, tag="ps", name=f"sts{_i}") for _i in range(2)]
                for j in range(3):
                    for hi in range(2):
                        po = hi * 64
                        nc.tensor.matmul(
                            sts[hi][:, j, :],
                            kt_sb[po : po + 64, hp, (t + j) * P : (t + j + 1) * P],
                            qt_sb[po : po + 64, hp, t * P : (t + 1) * P],
                            start=True,
                            stop=True,
                        )
                for hi in range(2):
                    h = 2 * hp + hi
                    st_ps = sts[hi]
                    exp_t = work.tile([P, 3, P], mybir.dt.float32, tag="exp")
                    for j in range(3):
                        nc.scalar.activation(
                            exp_t[:, j, :],
                            st_ps[:, j, :],
                            func=mybir.ActivationFunctionType.Exp,
                            bias=ab_sb[:, h * 3 + j : h * 3 + j + 1],
                            scale=1.0,
                        )
                    pt = work.tile([P, 3, P], bf16, tag="pt")
                    nc.vector.tensor_tensor(
                        pt[:],
                        exp_t[:],
                        mask_sb[:, t * 3 : (t + 1) * 3, :],
                        mybir.AluOpType.mult,
                    )
                    o_ps = psum.tile([P, VCOLS], mybir.dt.float32, tag="ps")
                    for j in range(3):
                        nc.tensor.matmul(
                            o_ps[:],
                            pt[:, j, :],
                            v_sb[:, t + j, h * VCOLS : (h + 1) * VCOLS],
                            start=(j == 0),
                            stop=(j == 2),
                        )
                    rs = rspool.tile([P, 1], mybir.dt.float32, tag="rs")
                    nc.vector.reciprocal(rs[:], o_ps[:, D : D + 1])
                    nc.vector.tensor_scalar_mul(
                        attn_t[:, h * D : (h + 1) * D], o_ps[:, 0:D], rs[:]
                    )

            # transpose attn [q, c] -> attnT [c, q] for the output projection
            at_t = consts.tile([P, KI, P], bf16, tag=f"attnT_{t}")
            for ct in range(KI):
                tr_ps = psum.tile([P, P], bf16, tag="ps")
                nc.tensor.transpose(
                    tr_ps[:], attn_t[:, ct * P : (ct + 1) * P], ident[:]
                )
                nc.vector.tensor_copy(at_t[:, ct, :], tr_ps[:])

            fin = finals.tile([P, C], mybir.dt.float32, tag="fin")
            # both output chunks inside the ct loop: adjacent matmuls share
            # the stationary attnT tile (one LDWEIGHTS after dedup)
            pps = [psum.tile([P, CH], mybir.dt.float32, tag="ps", name=f"pps{_i}") for _i in range(2)]
            for ct in range(KI):
                for cc in range(2):
                    nc.tensor.matmul(
                        pps[cc][:],
                        at_t[:, ct, :],
                        pwt_sb[:, ct, cc * 512 : (cc + 1) * 512],
                        start=(ct == 0),
                        stop=(ct == KI - 1),
                    )
            for cc in range(2):
                nc.vector.tensor_tensor(
                    fin[:, cc * 512 : (cc + 1) * 512],
                    pps[cc][:],
                    pb_sb[:, cc * 512 : (cc + 1) * 512],
                    mybir.AluOpType.add,
                )
            nc.sync.dma_start(out=out[t * P : (t + 1) * P, :], in_=fin[:])

    _dedup_ldweights(nc)
    _split_excess_waits(nc)
    return nc


_NC_CACHE = None


def _get_nc() -> bass.Bass:
    global _NC_CACHE
    if _NC_CACHE is None:
        _NC_CACHE = build_nc()
    return _NC_CACHE


# ---------------------------------------------------------------------------
# Host side: shard, pre-transpose, cast; run SPMD; gather
# ---------------------------------------------------------------------------
def make_in_maps(x, qkv_w, qkv_b, proj_w, proj_b):
    x = np.asarray(x, np.float32)
    qkv_w = np.asarray(qkv_w, np.float32)
    qkv_b = np.asarray(qkv_b, np.float32)
    proj_w = np.asarray(proj_w, np.float32)
    proj_b = np.asarray(proj_b, np.float32)

    # fold the attention scale into the Q projection
    qkv_w = qkv_w.copy()
    qkv_b = qkv_b.copy()
    qkv_w[:C] *= SCALE
    qkv_b[:C] *= SCALE

    wt_np = np.ascontiguousarray(qkv_w.T).astype(BF16)
    pwt_np = np.ascontiguousarray(proj_w.T).astype(BF16)
    qkvb_np = np.ascontiguousarray(qkv_b[: 2 * C])
    vb_np = np.ascontiguousarray(qkv_b[2 * C :])
    pb_np = proj_b

    slopes = _alibi_slopes(H)
    jj = np.arange(3, dtype=np.float32)[:, None]
    pp = np.arange(P, dtype=np.float32)[None, :]
    ab_np = np.ascontiguousarray(
        slopes[:, None, None] * (jj * P + pp - CBIAS)[None]
    ).astype(np.float32)  # [H, 3, P]

    tt = np.arange(QT_TILES)[:, None, None, None]
    jj4 = np.arange(3)[None, :, None, None]
    kk = np.arange(P)[None, None, :, None]
    qq = np.arange(P)[None, None, None, :]
    dist = jj4 * P + kk - qq  # key_local - t*128 - q_local
    valid = (dist >= 1) & (dist <= WINDOW)
    valid0 = valid & ((tt * P + jj4 * P + kk) >= HALO)  # chunk 0: no past ctx

    mask_np = np.broadcast_to(valid.astype(BF16), (QT_TILES, 3, P, P))
    mask0_np = valid0.astype(BF16)

    in_maps = []
    for core in range(NCORES):
        b, c = divmod(core, NCHUNK)
        n0 = c * CH
        xh = np.zeros((ROWS, C), np.float32)
        lo = max(0, n0 - HALO)
        xh[HALO - (n0 - lo) :] = x[b, lo : n0 + CH]
        in_maps.append(
            {
                "xt": np.ascontiguousarray(xh.T).astype(BF16),
                "wt": wt_np,
                "pwt": pwt_np,
                "qkvb": qkvb_np,
                "vb": vb_np,
                "pb": pb_np,
                "mask": np.ascontiguousarray(mask0_np if c == 0 else mask_np),
                "ab": ab_np,
            }
        )
    return in_maps


def run(in_maps, trace=False, **kw):
    res = run_bass_kernel_spmd(
        _get_nc(), in_maps, core_ids=list(range(NCORES)), trace=trace, **kw
    )
    return res


def kernel(x, qkv_w, qkv_b, proj_w, proj_b):
    in_maps = make_in_maps(x, qkv_w, qkv_b, proj_w, proj_b)
    res = run(in_maps)
    out = np.empty((B, N, C), np.float32)
    for core in range(NCORES):
        b, c = divmod(core, NCHUNK)
        out[b, c * CH : (c + 1) * CH] = res.results[core]["out"]
    return out


# revision 28
# speedup vs baseline: 1.3516x; 1.1394x over previous
"""ALiBi sliding-window causal attention (B=2, N=2048, C=1024, H=16, D=64,
W=256) on 8 TRN2 NeuronCores.

Sharding: core = (batch b, sequence chunk c) over a 2x4 grid. Each core owns
512 queries and recomputes K/V for a 256-row halo, so the sliding-window
attention is fully local — no collectives. Matmuls run in bf16 with f32
accumulation; weights/x are pre-transposed and cast on the host.

Key trick: in the S^T = K·Q^T layout (keys on partitions), the ALiBi bias
slope_h*(j - i) splits into a per-key term (a per-partition scalar, fused into
the ScalarE exp as its bias operand) and a per-query term that is constant
along the softmax axis and therefore cancels in the normalization. The
window/causal mask is a multiplicative {0,1} tile applied by the f32->bf16
conversion multiply. The softmax denominator comes from a ones-column
appended to V.
"""

import contextlib
import math

import numpy as np
import ml_dtypes

import concourse.bass as bass
import concourse.bass_utils as bass_utils
import concourse.mybir as mybir
import concourse.tile as tile
from concourse.bass_utils import run_bass_kernel_spmd
from concourse.masks import make_identity
from concourse.vector_clock import ScopedClock

# ---------------------------------------------------------------------------
# Patch TileContext._drain_and_barrier: this container's walrus rejects >2 sem
# waits on a CTRL-class instruction ("Too many sync wait commands"), and the
# Tile kernel-tail drain aggregates one wait per live proc. Split the waits
# onto single-wait nop carriers that run just before the drain's barrier.
# ---------------------------------------------------------------------------
_MAX_DRAIN_WAITS = 1


def _patched_drain_and_barrier(self, tick_clock, wait_clock):
    nc = self.nc
    drain_inst = nc.sync.drain()
    wait_clock.add_sem_waits(
        drain_inst.ins, ScopedClock({None: tick_clock.global_clock})
    )
    si = drain_inst.ins.sync_info
    waits = list(si.on_wait) if (si is not None and si.on_wait) else []
    if len(waits) > _MAX_DRAIN_WAITS:
        ups = list(si.on_update) if (si is not None and si.on_update) else []
        drain_inst.ins.sync_info = mybir.SyncInfo(
            on_wait=waits[:_MAX_DRAIN_WAITS], on_update=ups
        )
        for i in range(_MAX_DRAIN_WAITS, len(waits), _MAX_DRAIN_WAITS):
            nop = nc.sync.nop(nofuse=True)
            nop.ins.sync_info = mybir.SyncInfo(
                on_wait=waits[i : i + _MAX_DRAIN_WAITS], on_update=[]
            )

    nc.all_engine_barrier()
    assert self.sems is not None
    popped = nc._tile_sem_poison_stack.pop()
    assert popped is self._sem_poison
    nc.clear_and_free_semaphores(list(self.sems.allocated().values()))
    nc.all_engine_barrier()


tile.TileContext._drain_and_barrier = _patched_drain_and_barrier

def _dedup_ldweights(nc: bass.Bass):
    """Tile's legalize emits one InstLdweights per matmul even when
    consecutive matmuls use the identical stationary operand. Each load costs
    ~107ns of serial PE time; drop exact-duplicate back-to-back loads (the PE
    array still holds the weights), folding any waits into the next matmul."""
    pe = mybir.EngineType.PE
    for f in nc.m.functions:
        for blk in f.blocks:
            insts = list(blk.instructions)
            new = []
            last_key = None
            pending_waits = []
            changed = False
            for inst in insts:
                if inst.engine != pe:
                    new.append(inst)
                    continue
                tn = type(inst).__name__
                if tn == "InstLdweights":
                    key = (
                        str(inst.ins[0]),
                        str(inst.tile_position),
                        str(inst.tile_size),
                        str(inst.is_transpose),
                        str(inst.perf_mode),
                    )
                    if key == last_key:
                        changed = True
                        si = inst.sync_info
                        if si is not None and si.on_wait:
                            pending_waits.extend(si.on_wait)
                        continue
                    last_key = key
                elif tn != "InstMatmult":
                    pass  # other PE insts don't touch the weight array
                if pending_waits:
                    si = inst.sync_info
                    waits = list(si.on_wait) if (si and si.on_wait) else []
                    ups = list(si.on_update) if (si and si.on_update) else []
                    inst.sync_info = mybir.SyncInfo(
                        on_wait=pending_waits + waits, on_update=ups
                    )
                    pending_waits = []
                new.append(inst)
            if changed:
                blk.instructions = new


_MAX_INST_WAITS = 1


def _split_excess_waits(nc: bass.Bass, max_waits: int = _MAX_INST_WAITS):
    """Walrus in this container rejects instructions carrying more than a
    couple of sem waits. Hoist excess waits onto same-engine nop carriers
    placed immediately before the offending instruction."""
    for f in nc.m.functions:
        for blk in f.blocks:
            snapshot = list(blk.instructions)
            new: list = []
            changed = False
            for inst in snapshot:
                si = inst.sync_info
                waits = list(si.on_wait) if (si is not None and si.on_wait) else []
                if len(waits) > max_waits:
                    changed = True
                    eng = nc.engines[inst.engine]
                    n_extra = len(waits) - max_waits
                    for i in range(0, n_extra, max_waits):
                        chunk = waits[i : min(i + max_waits, n_extra)]
                        nop = eng.nop(nofuse=True)
                        # eng.nop appended to the current bb; reclaim it
                        cur = nc.cur_bb.bb
                        cur.instructions = cur.instructions[:-1]
                        nop.ins.sync_info = mybir.SyncInfo(
                            on_wait=chunk, on_update=[]
                        )
                        new.append(nop.ins)
                    ups = list(si.on_update) if (si is not None and si.on_update) else []
                    inst.sync_info = mybir.SyncInfo(
                        on_wait=waits[n_extra:], on_update=ups
                    )
                new.append(inst)
            if changed:
                blk.instructions = new

# ---------------------------------------------------------------------------
# Problem constants (hardcoded per spec)
# ---------------------------------------------------------------------------
BF16 = ml_dtypes.bfloat16
B, N, C = 2, 2048, 1024
H, D = 16, 64
WINDOW = 256
SCALE = D ** -0.5
NCHUNK = 4  # sequence chunks per batch -> 2*4 = 8 cores
CH = N // NCHUNK  # 512 own rows per core
HALO = WINDOW  # 256 halo rows of K/V context
ROWS = CH + HALO  # 768 rows of x per core
QT_TILES = CH // 128  # 4 query tiles of 128
CBIAS = 320  # alibi per-key bias centering (overflow/underflow safe)
P = 128
KI = C // P  # 8 contraction tiles
CT3 = 3 * C // P  # 24 qkv output column tiles
VCOLS = D + 1  # per-head V columns incl. ones column
NCORES = 8


def _alibi_slopes(num_heads: int) -> np.ndarray:
    closest_pow2 = 2 ** math.floor(math.log2(num_heads))
    base = 2.0 ** (-(2.0 ** (-(math.log2(closest_pow2) - 3))))
    powers = np.arange(1, closest_pow2 + 1, dtype=np.float32)
    slopes = base ** powers
    if num_heads != closest_pow2:
        start = 2.0 ** (-(2.0 ** (-(math.log2(closest_pow2) - 3)) - 1))
        extra = np.linspace(start, base, num_heads - closest_pow2, dtype=np.float32)
        slopes = np.concatenate([slopes, extra])
    return slopes.astype(np.float32)


# ---------------------------------------------------------------------------
# Device program
# ---------------------------------------------------------------------------
def build_nc() -> bass.Bass:
    nc = bass.Bass()
    f32 = mybir.dt.float32
    bf16 = mybir.dt.bfloat16

    xt = nc.declare_dram_parameter("xt", [C, ROWS], bf16, isOutput=False)
    wt = nc.declare_dram_parameter("wt", [C, 3 * C], bf16, isOutput=False)
    pwt = nc.declare_dram_parameter("pwt", [C, C], bf16, isOutput=False)
    qkvb = nc.declare_dram_parameter("qkvb", [2 * C], f32, isOutput=False)
    vb = nc.declare_dram_parameter("vb", [C], f32, isOutput=False)
    pb = nc.declare_dram_parameter("pb", [C], f32, isOutput=False)
    mask = nc.declare_dram_parameter(
        "mask", [QT_TILES, 3, P, P], bf16, isOutput=False
    )
    ab = nc.declare_dram_parameter("ab", [H, 3, P], f32, isOutput=False)
    out = nc.declare_dram_parameter("out", [CH, C], f32, isOutput=True)

    with tile.TileContext(nc) as tc, contextlib.ExitStack() as ctx:
        consts = ctx.enter_context(tc.tile_pool(name="consts", bufs=1))
        work = ctx.enter_context(tc.tile_pool(name="work", bufs=3))
        rspool = ctx.enter_context(tc.tile_pool(name="rs", bufs=6))
        finals = ctx.enter_context(tc.tile_pool(name="finals", bufs=2))
        # one dynamic PSUM pool: every tile fits one 2KB bank, 8 banks total
        psum = ctx.enter_context(tc.tile_pool(name="psum", bufs=8, space="PSUM"))

        # ------------------------------- constant loads -------------------
        xt_sb = consts.tile([P, KI, ROWS], bf16, tag="xt")
        wt_sb = consts.tile([P, KI, 3 * C], bf16, tag="wt")
        pwt_sb = consts.tile([P, KI, C], bf16, tag="pwt")
        qkb_sb = consts.tile([P, 16], f32, tag="qkb")
        vb_sb = consts.tile([P, C], f32, tag="vb")
        pb_sb = consts.tile([P, C], f32, tag="pb")
        mask_sb = consts.tile([P, QT_TILES * 3, P], bf16, tag="mask")
        ab_sb = consts.tile([P, H * 3], f32, tag="ab")
        ident = consts.tile([P, P], bf16, tag="ident")

        xt_r = xt.rearrange("(ko p) n -> p ko n", p=P)
        wt_r = wt.rearrange("(ko p) c -> p ko c", p=P)
        pwt_r = pwt.rearrange("(ko p) c -> p ko c", p=P)
        for ki in range(KI):
            # Q weights + x interleaved per-ki so the Q matmul pipeline can
            # start as soon as the first ki chunk lands
            nc.sync.dma_start(out=wt_sb[:, ki, 0:C], in_=wt_r[:, ki, 0:C])
            nc.sync.dma_start(out=xt_sb[:, ki, :], in_=xt_r[:, ki, :])
        for ki in range(KI):
            nc.sync.dma_start(out=wt_sb[:, ki, C : 2 * C], in_=wt_r[:, ki, C : 2 * C])
        for ki in range(KI):
            nc.sync.dma_start(
                out=wt_sb[:, ki, 2 * C : 3 * C], in_=wt_r[:, ki, 2 * C : 3 * C]
            )
        for ki in range(KI):
            nc.sync.dma_start(out=pwt_sb[:, ki, :], in_=pwt_r[:, ki, :])
        nc.sync.dma_start(out=qkb_sb[:], in_=qkvb.rearrange("(t p) -> p t", p=P))
        nc.sync.dma_start(out=vb_sb[:], in_=vb[None, :].to_broadcast((P, C)))
        nc.sync.dma_start(out=pb_sb[:], in_=pb[None, :].to_broadcast((P, C)))
        nc.sync.dma_start(out=mask_sb[:], in_=mask.rearrange("t j p q -> p (t j) q"))
        nc.sync.dma_start(out=ab_sb[:], in_=ab.rearrange("h j p -> p (h j)"))
        make_identity(nc, ident)
        # pre-warm the ScalarE Exp table (~1.3us ACT_TABLE_LOAD) off the
        # attention critical path
        warm = work.tile([P, 1], mybir.dt.float32, tag="warm")
        nc.scalar.activation(
            warm[:], qkb_sb[:, 0:1], func=mybir.ActivationFunctionType.Exp
        )

        # ------------------------------- QKV projections ------------------
        # Q^T [c_out, 512 own rows] and K^T [c_out, 768 rows]: c_out on
        # partitions (lhsT = W^T tile), rows on free dim.
        qt_sb = consts.tile([P, KI, CH], bf16, tag="qt")
        kt_sb = consts.tile([P, KI, ROWS], bf16, tag="kt")
        v_sb = consts.tile([P, ROWS // P, H * VCOLS], bf16, tag="v")

        for ct in range(KI):  # Q: c_out tiles 0..7
            ps = psum.tile([P, CH], mybir.dt.float32, tag="ps")
            for ki in range(KI):
                nc.tensor.matmul(
                    ps[:],
                    wt_sb[:, ki, ct * P : (ct + 1) * P],
                    xt_sb[:, ki, HALO:ROWS],
                    start=(ki == 0),
                    stop=(ki == KI - 1),
                )
            nc.vector.tensor_scalar_add(qt_sb[:, ct, :], ps[:], qkb_sb[:, ct : ct + 1])

        for ct in range(KI):  # K: c_out tiles 8..15
            # both row chunks inside the ki loop: adjacent matmuls share the
            # stationary W tile (one LDWEIGHTS after dedup)
            ps0 = psum.tile([P, CH], mybir.dt.float32, tag="ps")
            ps1 = psum.tile([P, CH], mybir.dt.float32, tag="ps")
            for ki in range(KI):
                w_ap = wt_sb[:, ki, C + ct * P : C + (ct + 1) * P]
                nc.tensor.matmul(
                    ps0[:],
                    w_ap,
                    xt_sb[:, ki, 0:512],
                    start=(ki == 0),
                    stop=(ki == KI - 1),
                )
                nc.tensor.matmul(
                    ps1[:, :256],
                    w_ap,
                    xt_sb[:, ki, 512:ROWS],
                    start=(ki == 0),
                    stop=(ki == KI - 1),
                )
            nc.vector.tensor_scalar_add(
                kt_sb[:, ct, 0:512], ps0[:], qkb_sb[:, KI + ct : KI + ct + 1]
            )
            nc.vector.tensor_scalar_add(
                kt_sb[:, ct, 512:ROWS], ps1[:, :256], qkb_sb[:, KI + ct : KI + ct + 1]
            )

        # V in natural [rows, c_v] layout (rows on partitions): lhsT = x^T
        # tile, rhs = W^T v-columns. Interleave a ones column per head for the
        # softmax denominator.
        for hcol in range(H):
            nc.vector.memset(v_sb[:, :, hcol * VCOLS + D : hcol * VCOLS + D + 1], 1.0)
        v_view = v_sb.rearrange("p r (h c) -> p r h c", c=VCOLS)
        for rb in range(ROWS // P):
            # both c_v chunks inside the ki loop: adjacent matmuls share the
            # stationary x^T tile (one LDWEIGHTS after dedup)
            vps = [psum.tile([P, CH], mybir.dt.float32, tag="ps", name=f"vps{_i}") for _i in range(2)]
            for ki in range(KI):
                for cc in range(2):
                    nc.tensor.matmul(
                        vps[cc][:],
                        xt_sb[:, ki, rb * P : (rb + 1) * P],
                        wt_sb[:, ki, 2 * C + cc * 512 : 2 * C + (cc + 1) * 512],
                        start=(ki == 0),
                        stop=(ki == KI - 1),
                    )
            for cc in range(2):
                nc.vector.tensor_tensor(
                    v_view[:, rb, cc * 8 : (cc + 1) * 8, 0:D],
                    vps[cc][:].rearrange("p (h c) -> p h c", c=D),
                    vb_sb[:, cc * 512 : (cc + 1) * 512].rearrange(
                        "p (h c) -> p h c", c=D
                    ),
                    mybir.AluOpType.add,
                )

        # ------------------------------- attention + proj -----------------
        for t in range(QT_TILES):
            attn_t = consts.tile([P, C], bf16, tag=f"attn_{t}")
            for hp in range(H // 2):
                # the S^T matmuls of a head pair contract on disjoint PE
                # row-groups (partitions 0-63 / 64-127); interleaving them
                # lets the PE pull each LDWEIGHTS ahead of the in-flight
                # matmul of the other head
                sts = [psum.tile([P, 3, P], mybir.dt.float32, tag="ps", name=f"sts{_i}") for _i in range(2)]
                for j in range(3):
                    for hi in range(2):
                        po = hi * 64
                        nc.tensor.matmul(
                            sts[hi][:, j, :],
                            kt_sb[po : po + 64, hp, (t + j) * P : (t + j + 1) * P],
                            qt_sb[po : po + 64, hp, t * P : (t + 1) * P],
                            start=True,
                            stop=True,
                        )
                for hi in range(2):
                    h = 2 * hp + hi
                    st_ps = sts[hi]
                    exp_t = work.tile([P, 3, P], mybir.dt.float32, tag="exp")
                    for j in range(3):
                        nc.scalar.activation(
                            exp_t[:, j, :],
                            st_ps[:, j, :],
                            func=mybir.ActivationFunctionType.Exp,
                            bias=ab_sb[:, h * 3 + j : h * 3 + j + 1],
                            scale=1.0,
                        )
                    pt = work.tile([P, 3, P], bf16, tag="pt")
                    nc.vector.tensor_tensor(
                        pt[:],
                        exp_t[:],
                        mask_sb[:, t * 3 : (t + 1) * 3, :],
                        mybir.AluOpType.mult,
                    )
                    o_ps = psum.tile([P, VCOLS], mybir.dt.float32, tag="ps")
                    for j in range(3):
                        nc.tensor.matmul(
                            o_ps[:],
                            pt[:, j, :],
                            v_sb[:, t + j, h * VCOLS : (h + 1) * VCOLS],
                            start=(j == 0),
                            stop=(j == 2),
                        )
                    rs = rspool.tile([P, 1], mybir.dt.float32, tag="rs")
                    nc.vector.reciprocal(rs[:], o_ps[:, D : D + 1])
                    nc.vector.tensor_scalar_mul(
                        attn_t[:, h * D : (h + 1) * D], o_ps[:, 0:D], rs[:]
                    )

            # transpose attn [q, c] -> attnT [c, q] for the output projection
            at_t = consts.tile([P, KI, P], bf16, tag=f"attnT_{t}")
            for ct in range(KI):
                tr_ps = psum.tile([P, P], bf16, tag="ps")
                nc.tensor.transpose(
                    tr_ps[:], attn_t[:, ct * P : (ct + 1) * P], ident[:]
                )
                nc.vector.tensor_copy(at_t[:, ct, :], tr_ps[:])

            fin = finals.tile([P, C], mybir.dt.float32, tag="fin")
            # both output chunks inside the ct loop: adjacent matmuls share
            # the stationary attnT tile (one LDWEIGHTS after dedup)
            pps = [psum.tile([P, CH], mybir.dt.float32, tag="ps", name=f"pps{_i}") for _i in range(2)]
            for ct in range(KI):
                for cc in range(2):
                    nc.tensor.matmul(
                        pps[cc][:],
                        at_t[:, ct, :],
                        pwt_sb[:, ct, cc * 512 : (cc + 1) * 512],
                        start=(ct == 0),
                        stop=(ct == KI - 1),
                    )
            for cc in range(2):
                nc.vector.tensor_tensor(
                    fin[:, cc * 512 : (cc + 1) * 512],
                    pps[cc][:],
                    pb_sb[:, cc * 512 : (cc + 1) * 512],
                    mybir.AluOpType.add,
                )
            nc.sync.dma_start(out=out[t * P : (t + 1) * P, :], in_=fin[:])

    _dedup_ldweights(nc)
    _split_excess_waits(nc)
    return nc


_NC_CACHE = None


def _get_nc() -> bass.Bass:
    global _NC_CACHE
    if _NC_CACHE is None:
        _NC_CACHE = build_nc()
    return _NC_CACHE


# ---------------------------------------------------------------------------
# Host side: shard, pre-transpose, cast; run SPMD; gather
# ---------------------------------------------------------------------------
def make_in_maps(x, qkv_w, qkv_b, proj_w, proj_b):
    x = np.asarray(x, np.float32)
    qkv_w = np.asarray(qkv_w, np.float32)
    qkv_b = np.asarray(qkv_b, np.float32)
    proj_w = np.asarray(proj_w, np.float32)
    proj_b = np.asarray(proj_b, np.float32)

    # fold the attention scale into the Q projection
    qkv_w = qkv_w.copy()
    qkv_b = qkv_b.copy()
    qkv_w[:C] *= SCALE
    qkv_b[:C] *= SCALE

    wt_np = np.ascontiguousarray(qkv_w.T).astype(BF16)
    pwt_np = np.ascontiguousarray(proj_w.T).astype(BF16)
    qkvb_np = np.ascontiguousarray(qkv_b[: 2 * C])
    vb_np = np.ascontiguousarray(qkv_b[2 * C :])
    pb_np = proj_b

    slopes = _alibi_slopes(H)
    jj = np.arange(3, dtype=np.float32)[:, None]
    pp = np.arange(P, dtype=np.float32)[None, :]
    ab_np = np.ascontiguousarray(
        slopes[:, None, None] * (jj * P + pp - CBIAS)[None]
    ).astype(np.float32)  # [H, 3, P]

    tt = np.arange(QT_TILES)[:, None, None, None]
    jj4 = np.arange(3)[None, :, None, None]
    kk = np.arange(P)[None, None, :, None]
    qq = np.arange(P)[None, None, None, :]
    dist = jj4 * P + kk - qq  # key_local - t*128 - q_local
    valid = (dist >= 1) & (dist <= WINDOW)
    valid0 = valid & ((tt * P + jj4 * P + kk) >= HALO)  # chunk 0: no past ctx

    mask_np = np.broadcast_to(valid.astype(BF16), (QT_TILES, 3, P, P))
    mask0_np = valid0.astype(BF16)

    in_maps = []
    for core in range(NCORES):
        b, c = divmod(core, NCHUNK)
        n0 = c * CH
        xh = np.zeros((ROWS, C), np.float32)
        lo = max(0, n0 - HALO)
        xh[HALO - (n0 - lo) :] = x[b, lo : n0 + CH]
        in_maps.append(
            {
                "xt": np.ascontiguousarray(xh.T).astype(BF16),
                "wt": wt_np,
                "pwt": pwt_np,
                "qkvb": qkvb_np,
                "vb": vb_np,
                "pb": pb_np,
                "mask": np.ascontiguousarray(mask0_np if c == 0 else mask_np),
                "ab": ab_np,
            }
        )
    return in_maps


def run(in_maps, trace=False, **kw):
    res = run_bass_kernel_spmd(
        _get_nc(), in_maps, core_ids=list(range(NCORES)), trace=trace, **kw
    )
    return res


def kernel(x, qkv_w, qkv_b, proj_w, proj_b):
    in_maps = make_in_maps(x, qkv_w, qkv_b, proj_w, proj_b)
    res = run(in_maps)
    out = np.empty((B, N, C), np.float32)
    for core in range(NCORES):
        b, c = divmod(core, NCHUNK)
        out[b, c * CH : (c + 1) * CH] = res.results[core]["out"]
    return out


# revision 31
# speedup vs baseline: 1.8016x; 1.3329x over previous
"""ALiBi sliding-window causal attention (B=2, N=2048, C=1024, H=16, D=64,
W=256) on 8 TRN2 NeuronCores.

Sharding: core = (batch b, sequence chunk c) over a 2x4 grid. Each core owns
512 queries and recomputes K/V for a 256-row halo, so the sliding-window
attention is fully local — no collectives. Matmuls run in bf16 with f32
accumulation; weights/x are pre-transposed and cast on the host.

Key trick: in the S^T = K·Q^T layout (keys on partitions), the ALiBi bias
slope_h*(j - i) splits into a per-key term (a per-partition scalar, fused into
the ScalarE exp as its bias operand) and a per-query term that is constant
along the softmax axis and therefore cancels in the normalization. The
window/causal mask is a multiplicative {0,1} tile applied by the f32->bf16
conversion multiply. The softmax denominator comes from a ones-column
appended to V.
"""

import contextlib
import math

import numpy as np
import ml_dtypes

import concourse.bass as bass
import concourse.bass_utils as bass_utils
import concourse.mybir as mybir
import concourse.tile as tile
from concourse.bass_utils import run_bass_kernel_spmd
from concourse.masks import make_identity
from concourse.vector_clock import ScopedClock

# ---------------------------------------------------------------------------
# Patch TileContext._drain_and_barrier: this container's walrus rejects >2 sem
# waits on a CTRL-class instruction ("Too many sync wait commands"), and the
# Tile kernel-tail drain aggregates one wait per live proc. Split the waits
# onto single-wait nop carriers that run just before the drain's barrier.
# ---------------------------------------------------------------------------
_MAX_DRAIN_WAITS = 1


def _patched_drain_and_barrier(self, tick_clock, wait_clock):
    nc = self.nc
    drain_inst = nc.sync.drain()
    wait_clock.add_sem_waits(
        drain_inst.ins, ScopedClock({None: tick_clock.global_clock})
    )
    si = drain_inst.ins.sync_info
    waits = list(si.on_wait) if (si is not None and si.on_wait) else []
    if len(waits) > _MAX_DRAIN_WAITS:
        ups = list(si.on_update) if (si is not None and si.on_update) else []
        drain_inst.ins.sync_info = mybir.SyncInfo(
            on_wait=waits[:_MAX_DRAIN_WAITS], on_update=ups
        )
        for i in range(_MAX_DRAIN_WAITS, len(waits), _MAX_DRAIN_WAITS):
            nop = nc.sync.nop(nofuse=True)
            nop.ins.sync_info = mybir.SyncInfo(
                on_wait=waits[i : i + _MAX_DRAIN_WAITS], on_update=[]
            )

    nc.all_engine_barrier()
    assert self.sems is not None
    popped = nc._tile_sem_poison_stack.pop()
    assert popped is self._sem_poison
    nc.clear_and_free_semaphores(list(self.sems.allocated().values()))
    nc.all_engine_barrier()


tile.TileContext._drain_and_barrier = _patched_drain_and_barrier

def _dedup_ldweights(nc: bass.Bass):
    """Tile's legalize emits one InstLdweights per matmul even when
    consecutive matmuls use the identical stationary operand. Each load costs
    ~107ns of serial PE time; drop exact-duplicate back-to-back loads (the PE
    array still holds the weights), folding any waits into the next matmul."""
    pe = mybir.EngineType.PE
    for f in nc.m.functions:
        for blk in f.blocks:
            insts = list(blk.instructions)
            new = []
            last_key = None
            pending_waits = []
            changed = False
            for inst in insts:
                if inst.engine != pe:
                    new.append(inst)
                    continue
                tn = type(inst).__name__
                if tn == "InstLdweights":
                    key = (
                        str(inst.ins[0]),
                        str(inst.tile_position),
                        str(inst.tile_size),
                        str(inst.is_transpose),
                        str(inst.perf_mode),
                    )
                    if key == last_key:
                        changed = True
                        si = inst.sync_info
                        if si is not None and si.on_wait:
                            pending_waits.extend(si.on_wait)
                        continue
                    last_key = key
                elif tn != "InstMatmult":
                    pass  # other PE insts don't touch the weight array
                if pending_waits:
                    si = inst.sync_info
                    waits = list(si.on_wait) if (si and si.on_wait) else []
                    ups = list(si.on_update) if (si and si.on_update) else []
                    inst.sync_info = mybir.SyncInfo(
                        on_wait=pending_waits + waits, on_update=ups
                    )
                    pending_waits = []
                new.append(inst)
            if changed:
                blk.instructions = new


_MAX_INST_WAITS = 1


def _split_excess_waits(nc: bass.Bass, max_waits: int = _MAX_INST_WAITS):
    """Walrus in this container rejects instructions carrying more than a
    couple of sem waits. Hoist excess waits onto same-engine nop carriers
    placed immediately before the offending instruction."""
    for f in nc.m.functions:
        for blk in f.blocks:
            snapshot = list(blk.instructions)
            new: list = []
            changed = False
            for inst in snapshot:
                si = inst.sync_info
                waits = list(si.on_wait) if (si is not None and si.on_wait) else []
                if len(waits) > max_waits:
                    changed = True
                    eng = nc.engines[inst.engine]
                    n_extra = len(waits) - max_waits
                    for i in range(0, n_extra, max_waits):
                        chunk = waits[i : min(i + max_waits, n_extra)]
                        nop = eng.nop(nofuse=True)
                        # eng.nop appended to the current bb; reclaim it
                        cur = nc.cur_bb.bb
                        cur.instructions = cur.instructions[:-1]
                        nop.ins.sync_info = mybir.SyncInfo(
                            on_wait=chunk, on_update=[]
                        )
                        new.append(nop.ins)
                    ups = list(si.on_update) if (si is not None and si.on_update) else []
                    inst.sync_info = mybir.SyncInfo(
                        on_wait=waits[n_extra:], on_update=ups
                    )
                new.append(inst)
            if changed:
                blk.instructions = new

# ---------------------------------------------------------------------------
# Problem constants (hardcoded per spec)
# ---------------------------------------------------------------------------
BF16 = ml_dtypes.bfloat16
B, N, C = 2, 2048, 1024
H, D = 16, 64
WINDOW = 256
SCALE = D ** -0.5
NCHUNK = 4  # sequence chunks per batch -> 2*4 = 8 cores
CH = N // NCHUNK  # 512 own rows per core
HALO = WINDOW  # 256 halo rows of K/V context
ROWS = CH + HALO  # 768 rows of x per core
QT_TILES = CH // 128  # 4 query tiles of 128
CBIAS = 320  # alibi per-key bias centering (overflow/underflow safe)
P = 128
KI = C // P  # 8 contraction tiles
CT3 = 3 * C // P  # 24 qkv output column tiles
VCOLS = D + 1  # per-head V columns incl. ones column
NCORES = 8


def _alibi_slopes(num_heads: int) -> np.ndarray:
    closest_pow2 = 2 ** math.floor(math.log2(num_heads))
    base = 2.0 ** (-(2.0 ** (-(math.log2(closest_pow2) - 3))))
    powers = np.arange(1, closest_pow2 + 1, dtype=np.float32)
    slopes = base ** powers
    if num_heads != closest_pow2:
        start = 2.0 ** (-(2.0 ** (-(math.log2(closest_pow2) - 3)) - 1))
        extra = np.linspace(start, base, num_heads - closest_pow2, dtype=np.float32)
        slopes = np.concatenate([slopes, extra])
    return slopes.astype(np.float32)


# ---------------------------------------------------------------------------
# Device program
# ---------------------------------------------------------------------------
def build_nc() -> bass.Bass:
    nc = bass.Bass()
    f32 = mybir.dt.float32
    bf16 = mybir.dt.bfloat16

    xt = nc.declare_dram_parameter("xt", [C, ROWS], bf16, isOutput=False)
    wt = nc.declare_dram_parameter("wt", [C, 3 * C], bf16, isOutput=False)
    pwt = nc.declare_dram_parameter("pwt", [C, C], bf16, isOutput=False)
    qkvb = nc.declare_dram_parameter("qkvb", [2 * C], f32, isOutput=False)
    vb = nc.declare_dram_parameter("vb", [C], f32, isOutput=False)
    pb = nc.declare_dram_parameter("pb", [C], f32, isOutput=False)
    mask = nc.declare_dram_parameter(
        "mask", [QT_TILES, 3, P, P], bf16, isOutput=False
    )
    ab = nc.declare_dram_parameter("ab", [H, 3, P], f32, isOutput=False)
    out = nc.declare_dram_parameter("out", [CH, C], f32, isOutput=True)

    with tile.TileContext(nc) as tc, contextlib.ExitStack() as ctx:
        consts = ctx.enter_context(tc.tile_pool(name="consts", bufs=1))
        work = ctx.enter_context(tc.tile_pool(name="work", bufs=3))
        rspool = ctx.enter_context(tc.tile_pool(name="rs", bufs=6))
        finals = ctx.enter_context(tc.tile_pool(name="finals", bufs=2))
        # one dynamic PSUM pool: every tile fits one 2KB bank, 8 banks total
        psum = ctx.enter_context(tc.tile_pool(name="psum", bufs=8, space="PSUM"))

        # ------------------------------- constant loads -------------------
        xt_sb = consts.tile([P, KI, ROWS], bf16, tag="xt")
        wt_sb = consts.tile([P, KI, 3 * C], bf16, tag="wt")
        pwt_sb = consts.tile([P, KI, C], bf16, tag="pwt")
        qkb_sb = consts.tile([P, 16], f32, tag="qkb")
        vb_sb = consts.tile([P, C], f32, tag="vb")
        pb_sb = consts.tile([P, C], f32, tag="pb")
        mask_sb = consts.tile([P, QT_TILES * 3, P], bf16, tag="mask")
        ab_sb = consts.tile([P, H * 3], f32, tag="ab")
        ident = consts.tile([P, P], bf16, tag="ident")

        xt_r = xt.rearrange("(ko p) n -> p ko n", p=P)
        wt_r = wt.rearrange("(ko p) c -> p ko c", p=P)
        pwt_r = pwt.rearrange("(ko p) c -> p ko c", p=P)
        # DMA order = consumption order: V weights + x first (V projection is
        # the first compute phase and pipelines per-ki with these arrivals),
        # then Q weights, K weights, attention constants, proj weights.
        nc.sync.dma_start(out=vb_sb[:], in_=vb[None, :].to_broadcast((P, C)))
        for ki in range(KI):
            nc.sync.dma_start(
                out=wt_sb[:, ki, 2 * C : 3 * C], in_=wt_r[:, ki, 2 * C : 3 * C]
            )
            nc.sync.dma_start(out=xt_sb[:, ki, :], in_=xt_r[:, ki, :])
        nc.sync.dma_start(out=qkb_sb[:], in_=qkvb.rearrange("(t p) -> p t", p=P))
        for ki in range(KI):
            nc.sync.dma_start(out=wt_sb[:, ki, 0:C], in_=wt_r[:, ki, 0:C])
        for ki in range(KI):
            nc.sync.dma_start(out=wt_sb[:, ki, C : 2 * C], in_=wt_r[:, ki, C : 2 * C])
        nc.sync.dma_start(out=mask_sb[:], in_=mask.rearrange("t j p q -> p (t j) q"))
        nc.sync.dma_start(out=ab_sb[:], in_=ab.rearrange("h j p -> p (h j)"))
        nc.sync.dma_start(out=pb_sb[:], in_=pb[None, :].to_broadcast((P, C)))
        for ki in range(KI):
            nc.sync.dma_start(out=pwt_sb[:, ki, :], in_=pwt_r[:, ki, :])
        make_identity(nc, ident)
        # pre-warm the ScalarE Exp table (~1.3us ACT_TABLE_LOAD) off the
        # attention critical path
        warm = work.tile([P, 1], mybir.dt.float32, tag="warm")
        nc.scalar.activation(
            warm[:], qkb_sb[:, 0:1], func=mybir.ActivationFunctionType.Exp
        )

        # ------------------------------- QKV projections ------------------
        # Q^T [c_out, 512 own rows] and K^T [c_out, 768 rows]: c_out on
        # partitions (lhsT = W^T tile), rows on free dim.
        qt_sb = consts.tile([P, KI, CH], bf16, tag="qt")
        kt_sb = consts.tile([P, KI, ROWS], bf16, tag="kt")
        v_sb = consts.tile([P, ROWS // P, H * VCOLS], bf16, tag="v")

        # V first: its weights+x arrive first, so its per-ki matmul pipeline
        # starts ~2us in; Q/K weights stream in while V computes.
        for hcol in range(H):
            nc.vector.memset(v_sb[:, :, hcol * VCOLS + D : hcol * VCOLS + D + 1], 1.0)
        v_view = v_sb.rearrange("p r (h c) -> p r h c", c=VCOLS)
        for rb in range(ROWS // P):
            # both c_v chunks inside the ki loop: adjacent matmuls share the
            # stationary x^T tile (one LDWEIGHTS after dedup)
            vps = [
                psum.tile([P, CH], mybir.dt.float32, tag="ps", name=f"vps{_i}")
                for _i in range(2)
            ]
            for ki in range(KI):
                for cc in range(2):
                    nc.tensor.matmul(
                        vps[cc][:],
                        xt_sb[:, ki, rb * P : (rb + 1) * P],
                        wt_sb[:, ki, 2 * C + cc * 512 : 2 * C + (cc + 1) * 512],
                        start=(ki == 0),
                        stop=(ki == KI - 1),
                    )
            for cc in range(2):
                nc.vector.tensor_tensor(
                    v_view[:, rb, cc * 8 : (cc + 1) * 8, 0:D],
                    vps[cc][:].rearrange("p (h c) -> p h c", c=D),
                    vb_sb[:, cc * 512 : (cc + 1) * 512].rearrange(
                        "p (h c) -> p h c", c=D
                    ),
                    mybir.AluOpType.add,
                )

        for ct in range(KI):  # Q: c_out tiles 0..7
            ps = psum.tile([P, CH], mybir.dt.float32, tag="ps")
            for ki in range(KI):
                nc.tensor.matmul(
                    ps[:],
                    wt_sb[:, ki, ct * P : (ct + 1) * P],
                    xt_sb[:, ki, HALO:ROWS],
                    start=(ki == 0),
                    stop=(ki == KI - 1),
                )
            nc.vector.tensor_scalar_add(qt_sb[:, ct, :], ps[:], qkb_sb[:, ct : ct + 1])

        for ct in range(KI):  # K: c_out tiles 8..15
            # both row chunks inside the ki loop: adjacent matmuls share the
            # stationary W tile (one LDWEIGHTS after dedup)
            ps0 = psum.tile([P, CH], mybir.dt.float32, tag="ps")
            ps1 = psum.tile([P, CH], mybir.dt.float32, tag="ps")
            for ki in range(KI):
                w_ap = wt_sb[:, ki, C + ct * P : C + (ct + 1) * P]
                nc.tensor.matmul(
                    ps0[:],
                    w_ap,
                    xt_sb[:, ki, 0:512],
                    start=(ki == 0),
                    stop=(ki == KI - 1),
                )
                nc.tensor.matmul(
                    ps1[:, :256],
                    w_ap,
                    xt_sb[:, ki, 512:ROWS],
                    start=(ki == 0),
                    stop=(ki == KI - 1),
                )
            nc.vector.tensor_scalar_add(
                kt_sb[:, ct, 0:512], ps0[:], qkb_sb[:, KI + ct : KI + ct + 1]
            )
            nc.vector.tensor_scalar_add(
                kt_sb[:, ct, 512:ROWS], ps1[:, :256], qkb_sb[:, KI + ct : KI + ct + 1]
            )

        # ------------------------------- attention + proj -----------------
        # Flat software-pipelined loop over (t, head-pair): iteration i emits
        # the S^T matmuls + exp of pair i, then the mask-mult / PV matmuls /
        # normalize of pair i-1. This keeps each engine's static FIFO free of
        # head-of-line blocking: when the PE reaches PV(i-1), its pt operand
        # was produced while the PE ran ST(i).
        attn_tiles = {}

        def emit_stage_a(t, hp):
            # the two heads' S^T matmuls contract on disjoint PE row-groups
            # (partitions 0-63 / 64-127); interleaving lets the PE pull each
            # LDWEIGHTS ahead of the in-flight matmul of the other head
            sts = [
                psum.tile([P, 3, P], mybir.dt.float32, tag="ps", name=f"sts{_i}")
                for _i in range(2)
            ]
            for j in range(3):
                for hi in range(2):
                    po = hi * 64
                    nc.tensor.matmul(
                        sts[hi][:, j, :],
                        kt_sb[po : po + 64, hp, (t + j) * P : (t + j + 1) * P],
                        qt_sb[po : po + 64, hp, t * P : (t + 1) * P],
                        start=True,
                        stop=True,
                    )
            exps = []
            for hi in range(2):
                h = 2 * hp + hi
                exp_t = work.tile([P, 3, P], mybir.dt.float32, tag="exp", name="exp")
                for j in range(3):
                    nc.scalar.activation(
                        exp_t[:, j, :],
                        sts[hi][:, j, :],
                        func=mybir.ActivationFunctionType.Exp,
                        bias=ab_sb[:, h * 3 + j : h * 3 + j + 1],
                        scale=1.0,
                    )
                exps.append(exp_t)
            return exps

        def emit_stage_b(t, hp, exps):
            attn_t = attn_tiles[t]
            for hi in range(2):
                h = 2 * hp + hi
                pt = work.tile([P, 3, P], bf16, tag="pt", name="pt")
                nc.vector.tensor_tensor(
                    pt[:],
                    exps[hi][:],
                    mask_sb[:, t * 3 : (t + 1) * 3, :],
                    mybir.AluOpType.mult,
                )
                o_ps = psum.tile([P, VCOLS], mybir.dt.float32, tag="ps", name="o_ps")
                for j in range(3):
                    nc.tensor.matmul(
                        o_ps[:],
                        pt[:, j, :],
                        v_sb[:, t + j, h * VCOLS : (h + 1) * VCOLS],
                        start=(j == 0),
                        stop=(j == 2),
                    )
                rs = rspool.tile([P, 1], mybir.dt.float32, tag="rs", name="rs")
                nc.vector.reciprocal(rs[:], o_ps[:, D : D + 1])
                nc.vector.tensor_scalar_mul(
                    attn_t[:, h * D : (h + 1) * D], o_ps[:, 0:D], rs[:]
                )

        def emit_tail(t):
            # transpose attn [q, c] -> attnT [c, q] for the output projection
            attn_t = attn_tiles[t]
            at_t = consts.tile([P, KI, P], bf16, tag=f"attnT_{t}", name=f"at_{t}")
            for ct in range(KI):
                tr_ps = psum.tile([P, P], bf16, tag="ps", name="tr_ps")
                nc.tensor.transpose(
                    tr_ps[:], attn_t[:, ct * P : (ct + 1) * P], ident[:]
                )
                nc.vector.tensor_copy(at_t[:, ct, :], tr_ps[:])

            fin = finals.tile([P, C], mybir.dt.float32, tag="fin", name="fin")
            # both output chunks inside the ct loop: adjacent matmuls share
            # the stationary attnT tile (one LDWEIGHTS after dedup)
            pps = [
                psum.tile([P, CH], mybir.dt.float32, tag="ps", name=f"pps{_i}")
                for _i in range(2)
            ]
            for ct in range(KI):
                for cc in range(2):
                    nc.tensor.matmul(
                        pps[cc][:],
                        at_t[:, ct, :],
                        pwt_sb[:, ct, cc * 512 : (cc + 1) * 512],
                        start=(ct == 0),
                        stop=(ct == KI - 1),
                    )
            for cc in range(2):
                nc.vector.tensor_tensor(
                    fin[:, cc * 512 : (cc + 1) * 512],
                    pps[cc][:],
                    pb_sb[:, cc * 512 : (cc + 1) * 512],
                    mybir.AluOpType.add,
                )
            nc.sync.dma_start(out=out[t * P : (t + 1) * P, :], in_=fin[:])

        HPAIRS = H // 2
        seq = [(t, hp) for t in range(QT_TILES) for hp in range(HPAIRS)]
        pending = None  # (t, hp, exps)
        for i, (t, hp) in enumerate(seq):
            if hp == 0:
                attn_tiles[t] = consts.tile([P, C], bf16, tag=f"attn_{t}", name=f"attn_{t}")
            exps = emit_stage_a(t, hp)
            if pending is not None:
                pt_, php, pexps = pending
                emit_stage_b(pt_, php, pexps)
                if php == HPAIRS - 1:
                    emit_tail(pt_)
            pending = (t, hp, exps)
        pt_, php, pexps = pending
        emit_stage_b(pt_, php, pexps)
        emit_tail(pt_)

    _dedup_ldweights(nc)
    _split_excess_waits(nc)
    return nc


_NC_CACHE = None


def _get_nc() -> bass.Bass:
    global _NC_CACHE
    if _NC_CACHE is None:
        _NC_CACHE = build_nc()
    return _NC_CACHE


# ---------------------------------------------------------------------------
# Host side: shard, pre-transpose, cast; run SPMD; gather
# ---------------------------------------------------------------------------
def make_in_maps(x, qkv_w, qkv_b, proj_w, proj_b):
    x = np.asarray(x, np.float32)
    qkv_w = np.asarray(qkv_w, np.float32)
    qkv_b = np.asarray(qkv_b, np.float32)
    proj_w = np.asarray(proj_w, np.float32)
    proj_b = np.asarray(proj_b, np.float32)

    # fold the attention scale into the Q projection
    qkv_w = qkv_w.copy()
    qkv_b = qkv_b.copy()
    qkv_w[:C] *= SCALE
    qkv_b[:C] *= SCALE

    wt_np = np.ascontiguousarray(qkv_w.T).astype(BF16)
    pwt_np = np.ascontiguousarray(proj_w.T).astype(BF16)
    qkvb_np = np.ascontiguousarray(qkv_b[: 2 * C])
    vb_np = np.ascontiguousarray(qkv_b[2 * C :])
    pb_np = proj_b

    slopes = _alibi_slopes(H)
    jj = np.arange(3, dtype=np.float32)[:, None]
    pp = np.arange(P, dtype=np.float32)[None, :]
    ab_np = np.ascontiguousarray(
        slopes[:, None, None] * (jj * P + pp - CBIAS)[None]
    ).astype(np.float32)  # [H, 3, P]

    tt = np.arange(QT_TILES)[:, None, None, None]
    jj4 = np.arange(3)[None, :, None, None]
    kk = np.arange(P)[None, None, :, None]
    qq = np.arange(P)[None, None, None, :]
    dist = jj4 * P + kk - qq  # key_local - t*128 - q_local
    valid = (dist >= 1) & (dist <= WINDOW)
    valid0 = valid & ((tt * P + jj4 * P + kk) >= HALO)  # chunk 0: no past ctx

    mask_np = np.broadcast_to(valid.astype(BF16), (QT_TILES, 3, P, P))
    mask0_np = valid0.astype(BF16)

    in_maps = []
    for core in range(NCORES):
        b, c = divmod(core, NCHUNK)
        n0 = c * CH
        xh = np.zeros((ROWS, C), np.float32)
        lo = max(0, n0 - HALO)
        xh[HALO - (n0 - lo) :] = x[b, lo : n0 + CH]
        in_maps.append(
            {
                "xt": np.ascontiguousarray(xh.T).astype(BF16),
                "wt": wt_np,
                "pwt": pwt_np,
                "qkvb": qkvb_np,
                "vb": vb_np,
                "pb": pb_np,
                "mask": np.ascontiguousarray(mask0_np if c == 0 else mask_np),
                "ab": ab_np,
            }
        )
    return in_maps


def run(in_maps, trace=False, **kw):
    res = run_bass_kernel_spmd(
        _get_nc(), in_maps, core_ids=list(range(NCORES)), trace=trace, **kw
    )
    return res


def kernel(x, qkv_w, qkv_b, proj_w, proj_b):
    in_maps = make_in_maps(x, qkv_w, qkv_b, proj_w, proj_b)
    res = run(in_maps)
    out = np.empty((B, N, C), np.float32)
    for core in range(NCORES):
        b, c = divmod(core, NCHUNK)
        out[b, c * CH : (c + 1) * CH] = res.results[core]["out"]
    return out
